# revision 1
# baseline (speedup 1.0000x reference)
"""Trainium2 Bass kernel for nn_AttentionOnDetail (sparse patch attention).

Data-parallel over batch B=8 across 8 NeuronCores; one batch per core.
The kernel is dependency-bound, not throughput-bound, so the design is
latency-focused:
  - Host-side prep inside kernel(): W_qkvg.T / W_out.T pre-transposed,
    cos/sin tables pre-permuted with tao folded in, rmsnorm(sink)*tao
    rows precomputed (rope at position 0 is identity), small tables
    packed into one DMA.
  - x tile DMAs issued first; patch stats (ACT square+accum, DVE dot)
    pipeline behind them; per-tile logits transposed into a PSUM row via
    PE so top-4 selection needs no DMA.
  - Top-4 via max8 / fused threshold-mask / max_index (ascending patch
    order); token gather in (tensor, patch, token) order via one
    indirect DMA so each tensor's rows are contiguous afterwards.
  - qkvg projection: 8 fp32r matmuls (1 cycle/row), bf16 staging copies
    split ACT/DVE, then just 2 clean SBUF->SBUF rearrange DMAs to
    s-major layout (the (T,p,t) gather order makes src rows r map to
    dst partition 4r+b exactly; no DRAM bounce).  The v/g DMA is gated
    behind the rope's stt so the scheduler keeps ACT table loads off
    the critical path.
  - q and k stacked on 128 partitions: rmsnorm+rope for both costs one
    set of full-width ops, split 5/8 DVE + 3/8 Pool (op cost scales
    with free size only).  Only Sqrt/Sigmoid/Exp tables are ever
    loaded, each exactly once, in the rope shadow.
  - attention runs as two head-group pipelines (bf16 matmuls, no
    row-max softmax, one exp per group); 1/den folds into the sigmoid
    gate; y is computed transposed (swapped matmul operands) and gated
    against the PE-transposed gate, feeding the output projection
    without a separate yg transpose chain.
"""

import sys
import numpy as np

for _p in ("/opt/trn_rl_repo",):
    if _p not in sys.path:
        sys.path.insert(0, _p)

import concourse.bass as bass
import concourse.bacc as bacc
import concourse.tile as tile
from concourse import mybir
from concourse.bass_utils import run_bass_kernel_spmd

F32 = mybir.dt.float32
F32R = mybir.dt.float32r
BF16 = mybir.dt.bfloat16
I32 = mybir.dt.int32
U32 = mybir.dt.uint32
U16 = mybir.dt.uint16
AF = mybir.ActivationFunctionType
ALU = mybir.AluOpType
AX = mybir.AxisListType

B, T, C, H, T0 = 8, 8192, 128, 8, 16
NP = T // T0          # 512 patches
PATCH = T0 * C        # 2048 elements per patch
S = 65                # sink + 64 selected tokens
NSEL = 64
EPS = 1.1920929e-07
SCALE = 1.0 / float(np.sqrt(np.float32(C)))
NEG_BIG = -1.0e30


def rap(t, apl, offset=0):
    """Raw AP over a tile/AP's storage, flat element strides."""
    base = t if isinstance(t, bass.AP) else t[:]
    return bass.AP(tensor=base.tensor, offset=base.offset + offset,
                   ap=[list(x) for x in apl])


def build_kernel(nc):
    xb = nc.dram_tensor("xb", [T, C], F32, kind="ExternalInput")
    pw = nc.dram_tensor("pw", [1, PATCH + 128], F32R, kind="ExternalInput")
    wqT_d = nc.dram_tensor("wqT_d", [C, 4 * C * H], F32R, kind="ExternalInput")
    woT_d = nc.dram_tensor("woT_d", [C, H, C], F32, kind="ExternalInput")
    tabs = nc.dram_tensor("tabs", [128, 401], F32, kind="ExternalInput")
    negio = nc.dram_tensor("negio", [1, NP], F32, kind="ExternalInput")
    sinkvb = nc.dram_tensor("sinkvb", [1, H * C], U16, kind="ExternalInput")
    ident = nc.dram_tensor("ident", [128, 128], F32, kind="ExternalInput")
    out = nc.dram_tensor("out", [NSEL, C], F32, kind="ExternalOutput")

    with tile.TileContext(nc) as tc:
        _emit(tc, nc, xb, pw, wqT_d, woT_d, tabs,
              negio, sinkvb, ident, out)
    return nc


def _emit(tc, nc, xb, pw, wqT_d, woT_d, tabs,
          negio, sinkvb, ident, out):
    import os
    LEVEL = int(os.environ.get("KLEVEL", "9"))
    from contextlib import ExitStack
    ctx = ExitStack()
    with ctx:
        const1 = ctx.enter_context(tc.tile_pool(name="const1", bufs=1))
        xpool = ctx.enter_context(tc.tile_pool(name="xpool", bufs=1))
        junkp = ctx.enter_context(tc.tile_pool(name="junkp", bufs=1))
        stat = ctx.enter_context(tc.tile_pool(name="stat", bufs=4))
        sb = ctx.enter_context(tc.tile_pool(name="sb", bufs=1))
        psall = ctx.enter_context(tc.tile_pool(name="psall", bufs=1,
                                               space="PSUM"))
        # one tile owning all 8 PSUM banks; regions choreographed manually
        PS = psall.tile([128, 4096], F32)
        # region plan (f32 cols):
        #   0:1024     qk stack (q rows 0:64, k rows 64:128); later att
        #              [65, 520] at 0:520; later out [64, 128]
        #   1024:2048  v rows 0:64; later pT staging [65, 260] at 1024:1284
        #   2048:3072  g rows 0:64
        #   2560:3584  y [65, 1024] (after sigg consumed g)
        #   3072:3584  staging A (pw bcast, x_selT, qnT)
        #   3584:4096  logits row (rows 0:1) / knT staging / ygT staging
        LROW = 3584

        # ---------------- pw/ones + ident first, then the x stream --------
        pwo_sb = const1.tile([1, PATCH + 128], F32R)
        nc.sync.dma_start(out=pwo_sb[:, :], in_=pw[:, :])
        pw_sb = pwo_sb[0:1, 0:PATCH]
        ones_t = pwo_sb[0:1, PATCH:PATCH + 128]
        ident_t = const1.tile([128, 128], F32)
        nc.sync.dma_start(out=ident_t[:, :], in_=ident[:, :])

        def xdma(i):
            xp = xpool.tile([128, PATCH], F32, tag=f"xp{i}")
            nc.sync.dma_start(
                out=xp[:, :],
                in_=rap(xb[:, :], [[PATCH, 128], [1, PATCH]],
                        offset=i * 128 * PATCH))
            return xp

        xps = [xdma(0)]
        eps_t = const1.tile([128, 1], F32)
        nc.vector.memset(eps_t[:, :], EPS)

        # ---------------- remaining x tiles (tile 3 in halves) ----------
        for i in (1, 2):
            xps.append(xdma(i))
        xp3 = xpool.tile([128, PATCH], F32, tag="xp3")
        for hh in range(2):
            nc.sync.dma_start(
                out=xp3[:, 1024 * hh:1024 * (hh + 1)],
                in_=rap(xb[:, :], [[PATCH, 128], [1, 1024]],
                        offset=3 * 128 * PATCH + 1024 * hh))
        xps.append(xp3)
        tabs_t = const1.tile([128, 401], F32)
        nc.sync.dma_start(out=tabs_t[:, :], in_=tabs[:, :])
        cosdup_t = tabs_t[:, 0:128]
        sinpm_t = tabs_t[:, 128:256]
        sinkTq_t = tabs_t[:, 256:264]
        sinkTk_t = tabs_t[:, 264:272]
        cmask_t = tabs_t[0:S, 272:337]
        sel16_t = tabs_t[0:5, 337:401]
        negio_t = const1.tile([1, NP], F32)
        nc.sync.dma_start(out=negio_t[:, :], in_=negio[:, :])
        rhs5 = const1.tile([5, 1], F32)
        nc.vector.memset(rhs5[:, :], 1.0)

        # v sink row: host-rounded bf16 bits straight into v_sb row 64
        v_sb = sb.tile([S, H, C], BF16, tag="v_sb")
        nc.sync.dma_start(
            out=v_sb[NSEL:S, :, :],
            in_=sinkvb[:, :].bitcast(BF16).rearrange(
                "p (h c) -> p h c", h=H))

        wqT = const1.tile([C, 4 * C * H], F32R)
        for wch in range(4):
            nc.sync.dma_start(out=wqT[:, 1024 * wch:1024 * (wch + 1)],
                              in_=wqT_d[:, 1024 * wch:1024 * (wch + 1)])
        woT = const1.tile([C, H, C], F32)


        # preload the sqrt activation table while ACT is idle
        dummy = stat.tile([1, 1], F32)
        nc.vector.memset(dummy[:, :], 1.0)
        nc.scalar.activation(out=dummy[:, :], in_=dummy[:, :],
                             func=AF.Sqrt)

        woTb = const1.tile([C, H, C], BF16)

        # pw broadcast to 128 partitions via K=1 matmul into staging banks
        # (two tiny warmups first lift PE off the cold p-state)
        nc.tensor.matmul(out=PS[0:128, 3071:3072], lhsT=ident_t[:, :],
                         rhs=ident_t[:, 0:1], start=True, stop=True)
        nc.tensor.matmul(out=PS[0:128, 3071:3072], lhsT=ident_t[:, :],
                         rhs=ident_t[:, 0:1], start=True, stop=True)
        pwB = const1.tile([128, PATCH], F32)
        for q4 in range(4):
            base = 1536 + 512 * q4
            nc.tensor.matmul(out=PS[:, base:base + 512],
                             lhsT=ones_t,
                             rhs=pwo_sb[0:1, 512 * q4:512 * (q4 + 1)],
                             start=True, stop=True)
            if q4 % 2 == 0:
                nc.scalar.copy(out=pwB[:, 512 * q4:512 * (q4 + 1)],
                               in_=PS[:, base:base + 512])
            else:
                nc.vector.tensor_copy(out=pwB[:, 512 * q4:512 * (q4 + 1)],
                                      in_=PS[:, base:base + 512])

        # ---------------- phase 1: per-patch stats ----------------
        junk = junkp.tile([128, PATCH], F32, tag="junk")
        junk2 = junkp.tile([128, PATCH], F32, tag="junk2")
        ss_c = stat.tile([128, 4], F32, tag="ss_c")
        dot_c = stat.tile([128, 4], F32, tag="dot_c")
        rs_c = stat.tile([128, 4], F32, tag="rs_c")
        logit_c = stat.tile([128, 4], F32, tag="logit_c")
        ss_h = stat.tile([128, 2], F32, tag="ss_h")
        dot_h = stat.tile([128, 2], F32, tag="dot_h")
        for i in range(4):
            xp = xps[i]
            if i < 3:
                nc.scalar.activation(out=junk[:, :], in_=xp[:, :],
                                     func=AF.Square,
                                     accum_out=ss_c[:, i:i + 1])
                nc.vector.scalar_tensor_tensor(
                    out=junk2[:, :], in0=xp[:, :], scalar=1.0, in1=pwB[:, :],
                    op0=ALU.mult, op1=ALU.mult,
                    accum_out=dot_c[:, i:i + 1])
            else:
                # tile 3 in halves so its stats overlap its own DMA
                for hh in range(2):
                    cs = slice(1024 * hh, 1024 * (hh + 1))
                    nc.scalar.activation(out=junk[:, cs], in_=xp[:, cs],
                                         func=AF.Square,
                                         accum_out=ss_h[:, hh:hh + 1])
                    nc.vector.scalar_tensor_tensor(
                        out=junk2[:, cs], in0=xp[:, cs], scalar=1.0,
                        in1=pwB[:, cs], op0=ALU.mult, op1=ALU.mult,
                        accum_out=dot_h[:, hh:hh + 1])
                nc.scalar.activation(out=ss_c[:, 3:4], in_=ss_h[:, 0:1],
                                     func=AF.Identity, bias=ss_h[:, 1:2])
                nc.vector.tensor_add(out=dot_c[:, 3:4], in0=dot_h[:, 0:1],
                                     in1=dot_h[:, 1:2])
            nc.scalar.activation(out=rs_c[:, i:i + 1], in_=ss_c[:, i:i + 1],
                                 func=AF.Sqrt, bias=eps_t[:, :],
                                 scale=1.0 / PATCH)
            nc.vector.reciprocal(out=rs_c[:, i:i + 1], in_=rs_c[:, i:i + 1])
            nc.vector.tensor_mul(logit_c[:, i:i + 1], dot_c[:, i:i + 1],
                                 rs_c[:, i:i + 1])
            # transpose this tile's logits column into the PSUM row
            nc.tensor.transpose(
                out=PS[0:1, LROW + 128 * i:LROW + 128 * (i + 1)],
                in_=logit_c[:, i:i + 1], identity=ident_t[:, :])

        if LEVEL == 1:
            lrow_sb = stat.tile([1, NP], F32, tag="lrow_sb")
            nc.vector.tensor_copy(out=lrow_sb[:, :],
                                  in_=PS[0:1, LROW:LROW + NP])
            for r in range(4):
                nc.sync.dma_start(out=out[r:r + 1, :],
                                  in_=lrow_sb[0:1, 128 * r:128 * (r + 1)])
            return

        # ---------------- top-4 selection (on the PSUM row) ----------------
        lrow = PS[0:1, LROW:LROW + NP]
        max8 = stat.tile([1, 8], F32, tag="max8")
        nc.vector.max(out=max8[:, :], in_=lrow)
        masked = stat.tile([1, NP], F32, tag="masked")
        nc.vector.scalar_tensor_tensor(
            out=masked[:, :], in0=lrow, scalar=max8[:, 3:4],
            in1=negio_t[:, :], op0=ALU.is_ge, op1=ALU.mult)
        mm8 = stat.tile([1, 8], F32, tag="mm8")
        nc.vector.max(out=mm8[:, :], in_=masked[:, :])
        idx8 = stat.tile([1, 8], U32, tag="idx8")
        nc.vector.max_index(out=idx8[:, :], in_max=mm8[:, :],
                            in_values=masked[:, :])
        idxf = stat.tile([1, 8], F32, tag="idxf")
        nc.vector.tensor_copy(out=idxf[:, :], in_=idx8[:, :])

        # patch-id column via PE transpose: [1,4] -> [4,1], then token
        # ids 16*I[p] + 4T + t in (T, p, t) row order via sel16
        nc.tensor.transpose(out=PS[0:4, 3582:3583], in_=idxf[0:1, 0:4],
                            identity=ident_t[0:1, 0:1])
        nc.scalar.copy(out=rhs5[0:4, :], in_=PS[0:4, 3582:3583])
        nc.tensor.matmul(out=PS[0:NSEL, 3583:3584], lhsT=sel16_t[:, :],
                         rhs=rhs5[:, :], start=True, stop=True)
        idc_i = stat.tile([NSEL, 1], I32, tag="idc_i")
        nc.vector.tensor_copy(out=idc_i[:, :], in_=PS[0:NSEL, 3583:3584])

        # gather the 64 tokens (row 16T+4p+t = token 16*I[p] + 4T + t)
        x_sel = sb.tile([NSEL, C], F32, tag="x_sel")
        nc.gpsimd.indirect_dma_start(
            out=x_sel[:, :], out_offset=None, in_=xb[:, :],
            in_offset=bass.IndirectOffsetOnAxis(ap=idc_i[:, 0:1], axis=0))


        if LEVEL == 2:
            nc.sync.dma_start(out=out[:, :], in_=x_sel[:, :])
            return

        # ---------------- qkvg projection ----------------
        nc.tensor.transpose(out=PS[0:128, 3072:3072 + NSEL], in_=x_sel[:, :],
                            identity=ident_t[0:NSEL, 0:NSEL])
        x_selT = sb.tile([C, NSEL], F32R, tag="x_selT")
        nc.scalar.copy(out=x_selT[:, :], in_=PS[:, 3072:3072 + NSEL])

        # qkvg[token, f] for the 64 gathered tokens -> PS rows 0:64
        for g in range(8):
            nc.tensor.matmul(out=PS[0:NSEL, 512 * g:512 * (g + 1)],
                             lhsT=x_selT[:, :],
                             rhs=wqT[:, 512 * g:512 * (g + 1)],
                             start=True, stop=True)
        qkvg_sb = sb.tile([NSEL, 4 * C * H], BF16, tag="qkvg_sb")
        nc.scalar.copy(out=qkvg_sb[:, 0:1024], in_=PS[0:NSEL, 0:1024])
        nc.vector.tensor_copy(out=qkvg_sb[:, 1024:2048],
                              in_=PS[0:NSEL, 1024:2048])
        nc.scalar.copy(out=qkvg_sb[:, 2048:3072], in_=PS[0:NSEL, 2048:3072])
        nc.vector.tensor_copy(out=qkvg_sb[:, 3072:4096],
                              in_=PS[0:NSEL, 3072:4096])

        # rearrange token-major -> s-major via SBUF->SBUF DMAs.
        # qkvg row 16T+4p+t (token 16*I[p]+4T+t), col (b,h,c) feeds
        # s-row 16p+4t+b of tensor T: per tensor the source rows are the
        # contiguous block 16T:16T+16 -> clean single-stride APs.
        qk = sb.tile([128, H, C], BF16, tag="qk")
        vg = sb.tile([128, H, C], BF16, tag="vg")
        FQ = 4 * C * H

        # one DMA per tensor-pair: src rows r=0:32 (q,k) in (T,p,t) order
        # land at dst partition 4r+b == the s-major qk stack exactly
        def rearr2(dst, pair):
            nc.sync.dma_start(
                out=dst[:, :, :],
                in_=rap(qkvg_sb[:, :], [[FQ, 2 * T0], [1024, 4], [1, 1024]],
                        offset=2 * T0 * pair * FQ))

        rearr2(qk, 0)

        # out-projection weights arrive late; the dummy write makes the DMA
        # wait for the gather so it cannot block the gather's transfer
        nc.vector.tensor_copy(out=woT[0:1, 0, 0:1], in_=x_sel[0:1, 0:1])
        nc.sync.dma_start(out=woT[:, :, :], in_=woT_d[:, :, :])
        nc.gpsimd.tensor_copy(out=woTb[:, :, :], in_=woT[:, :, :])

        if LEVEL == 3:
            q0 = sb.tile([NSEL, C], F32, tag="q0dbg")
            nc.vector.tensor_copy(out=q0[:, :], in_=qk[0:NSEL, 0, :])
            nc.sync.dma_start(out=out[:, :], in_=q0[:, :])
            return

        # ---------------- rmsnorm + rope on the qk stack ----------------
        ssq = sb.tile([128, H], F32, tag="ssq")
        sqj = junkp.tile([128, H, C], F32, tag="sqj")
        # heads 4:8 via ACT square+accum (ACT is idle here); 0:4 on DVE
        for h in range(4, 8):
            nc.scalar.activation(out=sqj[:, h, :], in_=qk[:, h, :],
                                 func=AF.Square, accum_out=ssq[:, h:h + 1])
        nc.vector.tensor_tensor(out=sqj[:, 0:4, :], in0=qk[:, 0:4, :],
                                in1=qk[:, 0:4, :], op=ALU.mult)
        # gate the v/g rearranges on the stt output so the scheduler keeps
        # sigmoid's table load behind the rope sqrt (value-preserving
        # corner write on the v/g source rows)
        zro = stat.tile([17, 1], F32, tag="zro")
        nc.vector.tensor_scalar_mul(zro[:, :], sqj[0:17, 0, 0:1], 0.0)
        corner = rap(qkvg_sb[:, :], [[FQ, 17], [1, 1]], offset=32 * FQ)
        nc.vector.tensor_scalar(out=corner, in0=corner,
                                scalar1=zro[:, 0:1], scalar2=None,
                                op0=ALU.add)
        rearr2(vg, 1)
        nc.gpsimd.tensor_copy(out=v_sb[0:NSEL, :, :], in_=vg[0:NSEL, :, :])
        nc.vector.tensor_reduce(out=ssq[:, 0:4], in_=sqj[:, 0:4, :],
                                axis=AX.X, op=ALU.add)
        rf = sb.tile([128, H], F32, tag="rf")
        nc.scalar.activation(out=rf[:, :], in_=ssq[:, :], func=AF.Sqrt,
                             bias=eps_t[:, :], scale=1.0 / C)
        nc.vector.reciprocal(out=rf[:, :], in_=rf[:, :])
        # sigmoid gate now, exp loads after: both ACT table loads land in
        # the rope shadow, and the softmax exps then run load-free
        sigg = sb.tile([NSEL, H, C], BF16, tag="sigg")
        nc.scalar.activation(out=sigg[:, :, :], in_=vg[NSEL:128, :, :],
                             func=AF.Sigmoid)
        qk1 = sb.tile([128, H, C], F32, tag="qk1")
        r1 = sb.tile([128, H, C], F32, tag="r1")
        r2 = sb.tile([128, H, C], F32, tag="r2")
        qkn = sb.tile([128, H, C], F32, tag="qkn")

        def hs(eng, hs0, hs1):
            hn = hs1 - hs0
            eng.tensor_tensor(
                out=qk1[:, hs0:hs1, :], in0=qk[:, hs0:hs1, :],
                in1=rf[:, hs0:hs1].rearrange("p (h a) -> p h a", a=1)
                    .to_broadcast([128, hn, C]), op=ALU.mult)
            eng.tensor_tensor(
                out=r1[:, hs0:hs1, :], in0=qk1[:, hs0:hs1, :],
                in1=cosdup_t[:, :].rearrange("p (a c) -> p a c", a=1)
                    .to_broadcast([128, hn, C]), op=ALU.mult)
            eng.tensor_tensor(
                out=r2[:, hs0:hs1, 0:64], in0=qk1[:, hs0:hs1, 64:128],
                in1=sinpm_t[:, 0:64].rearrange("p (a c) -> p a c", a=1)
                    .to_broadcast([128, hn, 64]), op=ALU.mult)
            eng.tensor_tensor(
                out=r2[:, hs0:hs1, 64:128], in0=qk1[:, hs0:hs1, 0:64],
                in1=sinpm_t[:, 64:128].rearrange("p (a c) -> p a c", a=1)
                    .to_broadcast([128, hn, 64]), op=ALU.mult)
            eng.tensor_add(out=qkn[:, hs0:hs1, :], in0=r1[:, hs0:hs1, :],
                           in1=r2[:, hs0:hs1, :])

        hs(nc.vector, 0, 5)
        hs(nc.gpsimd, 5, 8)

        if LEVEL == 4:
            qn32 = sb.tile([NSEL, C], F32, tag="qn32")
            nc.vector.tensor_copy(out=qn32[:, :], in_=qkn[0:NSEL, 0, :])
            nc.sync.dma_start(out=out[:, :], in_=qn32[:, :])
            return

        # ---------------- transposes to qnT / knT ----------------
        # per head-group so group-0 attention starts while group-1 is
        # still transposing; sink columns inserted up front
        qnT = sb.tile([C, H, S], BF16, tag="qnT")
        knT = sb.tile([C, H, S], BF16, tag="knT")
        nc.scalar.copy(out=rap(qnT[:, :, :], [[H * S, C], [S, H], [1, 1]],
                               offset=NSEL),
                       in_=sinkTq_t[:, :].rearrange("c (h a) -> c h a", a=1))
        nc.scalar.copy(out=rap(knT[:, :, :], [[H * S, C], [S, H], [1, 1]],
                               offset=NSEL),
                       in_=sinkTk_t[:, :].rearrange("c (h a) -> c h a", a=1))
        for g in range(2):
            for si, dstT in enumerate((qnT, knT)):
                base = 3072 + 256 * (2 * g + si)
                for j in range(4):
                    h = 4 * g + j
                    nc.tensor.transpose(
                        out=PS[:, base + NSEL * j:base + NSEL * (j + 1)],
                        in_=qkn[64 * si:64 * (si + 1), h, :],
                        identity=ident_t[64 * si:64 * si + NSEL,
                                         64 * si:64 * si + NSEL])
                dst = rap(dstT[:, :, :], [[H * S, C], [S, 4], [1, NSEL]],
                          offset=4 * g * S)
                nc.vector.tensor_copy(
                    out=dst, in_=PS[:, base:base + 256].rearrange(
                        "p (h s) -> p h s", h=4))

        # ---------------- attention ----------------
        # att head slots padded to 128 cols (matmul must not cross banks);
        # the whole tail runs as two independent head-group pipelines so
        # PE/DVE/ACT overlap across groups
        t0 = sb.tile([S, H, S], F32, tag="t0")
        p_sb = sb.tile([S, H, S], F32, tag="p_sb")
        den8 = sb.tile([S, H], F32, tag="den8")
        rden = sb.tile([S, H], F32, tag="rden")
        sigrd = sb.tile([NSEL, H, C], F32, tag="sigrd")
        pT = sb.tile([S, H, S], BF16, tag="pT")
        ygT = sb.tile([C, H, NSEL], BF16, tag="ygT")
        sgT_sb = sb.tile([C, H, NSEL], BF16, tag="sgT_sb")
        for g in range(2):
            hs = slice(4 * g, 4 * (g + 1))
            for h in range(4 * g, 4 * (g + 1)):
                nc.tensor.matmul(out=PS[0:S, C * h:C * h + S],
                                 lhsT=qnT[:, h, :], rhs=knT[:, h, :],
                                 start=True, stop=True)
            attg = rap(PS[:, :], [[4096, S], [C, 4], [1, S]],
                       offset=4 * g * C)
            nc.vector.tensor_tensor(
                out=t0[:, hs, :], in0=attg,
                in1=cmask_t[:, :].rearrange("s (a t) -> s a t", a=1)
                    .to_broadcast([S, 4, S]), op=ALU.add)
            nc.scalar.activation(out=p_sb[:, hs, :], in_=t0[:, hs, :],
                                 func=AF.Exp, scale=SCALE)
            nc.vector.tensor_reduce(out=den8[:, hs], in_=p_sb[:, hs, :],
                                    axis=AX.X, op=ALU.add)
            nc.vector.reciprocal(out=rden[:, hs], in_=den8[:, hs])
            # 1/den folds into the gate; pT/y consume UNNORMALIZED p
            nc.vector.tensor_tensor(
                out=sigrd[:, hs, :], in0=sigg[:, hs, :],
                in1=rden[0:NSEL, hs].rearrange("s (h a) -> s h a", a=1)
                    .to_broadcast([NSEL, 4, C]), op=ALU.mult)
            for j in range(4):
                nc.tensor.transpose(
                    out=PS[0:S, 1024 + 520 * g + S * j:
                           1024 + 520 * g + S * (j + 1)],
                    in_=p_sb[:, 4 * g + j, :], identity=ident_t[0:S, 0:S])
            nc.scalar.copy(
                out=pT[:, hs, :],
                in_=PS[0:S, 1024 + 520 * g:1024 + 520 * g + 4 * S]
                    .rearrange("p (a b) -> p a b", a=4))
            # yT = v^T @ p per head (swapped operands) -> [c, s] slots
            for h in range(4 * g, 4 * (g + 1)):
                nc.tensor.matmul(out=PS[0:C, 2560 + C * h:2560 + C * h + S],
                                 lhsT=v_sb[:, h, :], rhs=pT[:, h, :],
                                 start=True, stop=True)
            # transpose the gate into [c, h, s] during the same window
            for h in range(4 * g, 4 * (g + 1)):
                nc.tensor.transpose(
                    out=PS[:, LROW + NSEL * h:LROW + NSEL * (h + 1)],
                    in_=sigrd[:, h, :], identity=ident_t[0:NSEL, 0:NSEL])
            nc.scalar.copy(
                out=rap(sgT_sb[:, :, :],
                        [[H * NSEL, C], [NSEL, 4], [1, NSEL]],
                        offset=4 * g * NSEL),
                in_=PS[:, LROW + 256 * g:LROW + 256 * (g + 1)].rearrange(
                    "p (h s) -> p h s", h=4))
            yTg = rap(PS[:, :], [[4096, C], [C, 4], [1, NSEL]],
                      offset=2560 + 4 * g * C)
            nc.vector.tensor_tensor(
                out=rap(ygT[:, :, :], [[H * NSEL, C], [NSEL, 4], [1, NSEL]],
                        offset=4 * g * NSEL),
                in0=yTg,
                in1=rap(sgT_sb[:, :, :],
                        [[H * NSEL, C], [NSEL, 4], [1, NSEL]],
                        offset=4 * g * NSEL), op=ALU.mult)

        if LEVEL == 5:
            yg32 = sb.tile([NSEL, C], F32, tag="yg32")
            nc.vector.tensor_copy(out=yg32[:, :], in_=ygT[0:NSEL, 0, :])
            nc.sync.dma_start(out=out[:, :], in_=yg32[:, :])
            return

        # ---------------- output projection ----------------
        out_sb = sb.tile([NSEL, C], F32, tag="out_sb")
        out_ps = PS[0:NSEL, 0:128]
        for h in range(H):
            nc.tensor.matmul(out=out_ps, lhsT=ygT[:, h, :],
                             rhs=woTb[:, h, :], start=(h == 0),
                             stop=(h == H - 1))
        nc.scalar.copy(out=out_sb[:, :], in_=out_ps)
        nc.sync.dma_start(out=out[:, :], in_=out_sb[:, :])


def make_host_constants(inputs):
    """Host-side prep of tables derived from the (full) inputs."""
    cos = np.asarray(inputs["cos"]).reshape(S, 64).astype(np.float32)
    sin = np.asarray(inputs["sin"]).reshape(S, 64).astype(np.float32)
    sink = np.asarray(inputs["sink"]).reshape(H, C).astype(np.float32)
    tao = np.asarray(inputs["tao"]).astype(np.float32)
    wq = np.asarray(inputs["W_qkvg"]).astype(np.float32)
    wo = np.asarray(inputs["W_out"]).astype(np.float32)

    # partition p (0..63 in each half) holds position p+1; rows duplicated
    # for the q half (0:64) and k half (64:128)
    pos = np.arange(64) + 1
    cos_p = cos[pos]
    sin_p = sin[pos]
    cosdup = np.tile(np.concatenate([cos_p, cos_p], axis=1), (2, 1))
    sinpm = np.tile(np.concatenate([sin_p, -sin_p], axis=1), (2, 1))
    # tao folds into the rope tables: qn = (qk*rf)*cos' + swap(qk*rf)*sin'
    taocol = np.concatenate([np.full((64, 1), tao[0], np.float32),
                             np.full((64, 1), tao[1], np.float32)])
    cosdup = cosdup * taocol
    sinpm = sinpm * taocol

    # additive causal mask in s-major layout (row/col 64 = sink, pos 0)
    posf = np.where(np.arange(S) < NSEL, np.arange(S) + 1, 0)
    cmaskm = np.where(posf[None, :] <= posf[:, None], 0.0,
                      NEG_BIG).astype(np.float32)
    negio = (float(NP) - np.arange(NP, dtype=np.float32)).reshape(1, NP)

    # sink rows: rope at position 0 is identity; rmsnorm + tao on host
    sn = sink / np.sqrt((sink * sink).mean(axis=-1, keepdims=True) + EPS)
    sinkTq = np.ascontiguousarray((sn * tao[0]).T)
    sinkTk = np.ascontiguousarray((sn * tao[1]).T)
    # v sink row as bf16 bit pattern (round-to-nearest-even)
    f = sink.reshape(1, H * C).astype(np.float32)
    u = f.view(np.uint32)
    rounded = ((u + 0x7FFF + ((u >> 16) & 1)) >> 16).astype(np.uint16)
    sinkvb = np.ascontiguousarray(rounded)

    # token ids: row 16T+4p+t gathers token 16*I[p] + 4T + t
    # sel16[j, r] = 16*(j==p(r)) for j<4; sel16[4, r] = 4T(r) + t(r)
    sel16m = np.zeros((5, NSEL), np.float32)
    for Tn in range(4):
        for p in range(4):
            for t in range(4):
                r = 16 * Tn + 4 * p + t
                sel16m[p, r] = 16.0
                sel16m[4, r] = float(4 * Tn + t)

    wqT = np.ascontiguousarray(wq.T)
    woT = np.ascontiguousarray(wo.reshape(C, H, C).transpose(2, 1, 0))

    ident = np.eye(128, dtype=np.float32)
    tabs = np.zeros((128, 401), np.float32)
    tabs[:, 0:128] = cosdup
    tabs[:, 128:256] = sinpm
    tabs[:128, 256:264] = sinkTq
    tabs[:128, 264:272] = sinkTk
    tabs[:S, 272:337] = cmaskm
    tabs[:5, 337:401] = sel16m
    return dict(tabs=tabs, negio=negio, sinkvb=sinkvb,
                wqT_d=wqT, woT_d=woT, ident=ident)


_CACHE = {}


def get_nc():
    if "nc" not in _CACHE:
        nc = bacc.Bacc("TRN2", target_bir_lowering=False, debug=False,
                       num_devices=B)
        build_kernel(nc)
        nc.compile()
        _CACHE["nc"] = nc
    return _CACHE["nc"]


def make_in_maps(inputs):
    x = np.ascontiguousarray(inputs["x"], dtype=np.float32)
    pwv = np.concatenate(
        [np.asarray(inputs["patch_w"], np.float32).ravel(),
         np.ones(128, np.float32)]).reshape(1, PATCH + 128)
    consts = make_host_constants(inputs)
    in_maps = []
    for b in range(B):
        m = {"xb": np.ascontiguousarray(x[b]), "pw": pwv}
        m.update(consts)
        in_maps.append(m)
    return in_maps


def kernel(**inputs):
    nc = get_nc()
    in_maps = make_in_maps(inputs)
    res = run_bass_kernel_spmd(nc, in_maps, core_ids=list(range(B)))
    return np.stack([r["out"] for r in res.results], axis=0)


if __name__ == "__main__":
    nc = get_nc()
    print("build ok:", len(nc.m.functions[0].allocations), "allocations")



# revision 33
# speedup vs baseline: 1.0589x; 1.0589x over previous
"""Trainium2 Bass kernel for nn_AttentionOnDetail (sparse patch attention).

Data-parallel over batch B=8 across 8 NeuronCores; one batch per core.
v3 redesign (latency-focused; the kernel is dependency-bound):
  - x streamed first (x tile DMAs are the first SP descriptors; tile 3 in
    four 512-col chunks).  pw row + f32 ident ride gpsimd SWDGE so their
    transfers slip into the stream right after tile 0.
  - pw broadcast stays in PSUM (dot stt reads PSUM directly, no copies);
    PE warmup matmuls run on the eps tile at t~0.9 so everything after
    runs at peak p-state.
  - stats split across engines: ACT squares (t0-t2, 3a, 3b, 3d), DVE dots
    (t0-t2, 3c, 3d) + logit chain, Pool dots (3a, 3b) + square (3c).
  - patch logits use the monotone transform dot*|dot|/ms (no Sqrt); the
    selection runs once globally: top8 -> threshold mask * negio -> top8
    gives the ranks directly (idc matmul folds NP-v and the *16).
  - single ACT function set (Exp/Square/Copy, set 0) loaded once at t=0;
    rmsnorm rsqrt = Newton iteration on Pool (bit-trick seed), sigmoid
    via exp(-g), softmax exp with folded -6 bias so p fits fp16.
  - everything from the projection on runs in fp16 (W cast on host).
  - attention computed transposed (att_T = k^T q) with the causal mask
    preloaded into PSUM via an identity matmul; denominator broadcast to
    all partitions by a single ones[65,128] matmul; gate folded as
    yg = y / ((1+e^-g)*den) with DVE/Pool divide; output projection
    consumes yg^T directly.
"""

import sys
import numpy as np

for _p in ("/opt/trn_rl_repo",):
    if _p not in sys.path:
        sys.path.insert(0, _p)

import concourse.bass as bass
import concourse.bacc as bacc
import concourse.tile as tile
from concourse import mybir
from concourse.bass_utils import run_bass_kernel_spmd

F32 = mybir.dt.float32
F32R = mybir.dt.float32r
F16 = mybir.dt.float16
I32 = mybir.dt.int32
U32 = mybir.dt.uint32
AF = mybir.ActivationFunctionType
ALU = mybir.AluOpType
AX = mybir.AxisListType

B, T, C, H, T0 = 8, 8192, 128, 8, 16
NP = T // T0          # 512 patches
PATCH = T0 * C        # 2048 elements per patch
S = 65                # sink + 64 selected tokens
NSEL = 64
EPS = 1.1920929e-07
SCALE = 1.0 / float(np.sqrt(np.float32(C)))
EXPB = -6.0           # softmax exp bias; den-normalization cancels it
NEG_BIG = -60000.0    # additive causal mask (fp16-representable)
MAGIC = 0x5F3759DF    # fast-rsqrt seed

# tabs2 f32-column layout (fp16 payloads packed as pairs into f32 cols)
TB_SEL16 = 0          # sel16' f32 [5, 64]
TB_IDF16 = 64         # ident f16 [128, 128] -> 64 f32 cols
TB_COS = 128          # cosdup f16 [128, 128] -> 64
TB_SIN = 192          # sinpm f16 [128, 128] -> 64
TB_SINKQ = 256        # sinkTq f16 [128, 8] -> 4
TB_SINKK = 260        # sinkTk f16 [128, 8] -> 4
TB_CMASK = 264        # cmaskT f16 [65, 66] -> 33 (col 65 pad)
TB_NEGIO = 297        # negio f16 [1, 512] -> 256
TB_COLS = 553

# PSUM f32-column region plan (8 banks x 512 cols)
PB_PWB = 0            # pwB broadcast [128, 2048] (cols 0:2048), early only
PB_QNT16 = 0          # qkn transposes (f16 cols 0:1024 = f32 0:512)
PB_GT16 = 2048        # gT transposes (f16 cols 2048:2560 = f32 1024:1280)
PB_YT = 1536          # yT [128, (h,s)=512] cols 1536:2048 (bank 3)
PB_OUT = 3584         # out [64, 128] in bank 7 (logits row dead)
PB_ATT0 = 2048        # att_T group 0 [65, 260]
PB_ATT1 = 2560        # att_T group 1 [65, 260]
PB_XSELT = 3072       # x_selT staging [128, 64]
PB_DENB = 3072        # den broadcast [128, 512] (after x_selT dead)
PB_WARM = 3500        # warmup scratch col
PB_MM8 = 3582         # mm8 transpose column [4, 1]
PB_IDC = 3583         # sel16 matmul out [64, 1]
LROW = 3584           # logits row [1, 512] (rows 0:1)


def rap(t, apl, offset=0):
    """Raw AP over a tile/AP's storage, flat element strides."""
    base = t if isinstance(t, bass.AP) else t[:]
    return bass.AP(tensor=base.tensor, offset=base.offset + offset,
                   ap=[list(x) for x in apl])


def build_kernel(nc):
    xb = nc.dram_tensor("xb", [T, C], F32, kind="ExternalInput")
    pw = nc.dram_tensor("pw", [1, PATCH + 128], F32R, kind="ExternalInput")
    identd = nc.dram_tensor("identd", [128, 128], F32, kind="ExternalInput")
    tabs2 = nc.dram_tensor("tabs2", [128, TB_COLS], F32, kind="ExternalInput")
    wqT_d = nc.dram_tensor("wqT_d", [C, 4 * C * H], F16, kind="ExternalInput")
    woT_d = nc.dram_tensor("woT_d", [C, H, C], F16, kind="ExternalInput")
    sinkv = nc.dram_tensor("sinkv", [1, H * C], F16, kind="ExternalInput")
    out = nc.dram_tensor("out", [NSEL, C], F32, kind="ExternalOutput")

    with tile.TileContext(nc) as tc:
        _emit(tc, nc, xb, pw, identd, tabs2, wqT_d, woT_d, sinkv, out)
    return nc


def _emit(tc, nc, xb, pw, identd, tabs2_d, wqT_d, woT_d, sinkv, out):
    import os
    LEVEL = int(os.environ.get("KLEVEL", "9"))
    from contextlib import ExitStack
    ctx = ExitStack()
    with ctx:
        const1 = ctx.enter_context(tc.tile_pool(name="const1", bufs=1))
        xpool = ctx.enter_context(tc.tile_pool(name="xpool", bufs=1))
        junkp = ctx.enter_context(tc.tile_pool(name="junkp", bufs=1))
        stat = ctx.enter_context(tc.tile_pool(name="stat", bufs=4))
        sb = ctx.enter_context(tc.tile_pool(name="sb", bufs=1))
        psall = ctx.enter_context(tc.tile_pool(name="psall", bufs=1,
                                               space="PSUM"))
        PS = psall.tile([128, 4096], F32)
        PS16 = PS[:, :].bitcast(F16)  # [128, 8192] f16 view

        # ---------------- x stream first; pw/ident via gpsimd --------------
        def xdma(i):
            xp = xpool.tile([128, PATCH], F32, tag=f"xp{i}")
            nc.sync.dma_start(
                out=xp[:, :],
                in_=rap(xb[:, :], [[PATCH, 128], [1, PATCH]],
                        offset=i * 128 * PATCH))
            return xp

        xps = [xdma(0)]
        pwo_sb = const1.tile([1, PATCH + 128], F32R)
        nc.gpsimd.dma_start(out=pwo_sb[:, :], in_=pw[:, :])
        identf = const1.tile([128, 128], F32)
        nc.gpsimd.dma_start(out=identf[:, :], in_=identd[:, :])
        xps.append(xdma(1))
        xps.append(xdma(2))
        xp3 = xpool.tile([128, PATCH], F32, tag="xp3")
        for ch in range(4):
            nc.sync.dma_start(
                out=xp3[:, 512 * ch:512 * (ch + 1)],
                in_=rap(xb[:, :], [[PATCH, 128], [1, 512]],
                        offset=3 * 128 * PATCH + 512 * ch))
        xps.append(xp3)

        tabs2 = const1.tile([128, TB_COLS], F32)
        nc.sync.dma_start(out=tabs2[:, :], in_=tabs2_d[:, :])
        t2h = tabs2[:, :].bitcast(F16)  # [128, 2*TB_COLS] f16 view

        def h16(col_f32, ncols_f16, nrows=128):
            return rap(t2h, [[2 * TB_COLS, nrows], [1, ncols_f16]],
                       offset=2 * col_f32)

        identh_v = h16(TB_IDF16, 128)
        identh = None  # materialized below after tabs2 lands
        cosdup = h16(TB_COS, 128)
        sinkTq = h16(TB_SINKQ, 8)
        sinkTk = h16(TB_SINKK, 8)
        negio = h16(TB_NEGIO, 512, nrows=1)
        sel16 = tabs2[0:5, TB_SEL16:TB_SEL16 + 64]

        wqT = const1.tile([C, 4 * C * H], F16)
        for wch in range(4):
            nc.sync.dma_start(out=wqT[:, 1024 * wch:1024 * (wch + 1)],
                              in_=wqT_d[:, 1024 * wch:1024 * (wch + 1)])
        woTb = const1.tile([C, H, C], F16)
        nc.sync.dma_start(out=woTb[:, :, :], in_=woT_d[:, :, :])
        v_sb = sb.tile([S, H, C], F16, tag="v_sb")
        nc.sync.dma_start(
            out=v_sb[NSEL:S, :, :],
            in_=sinkv[:, :].rearrange("p (h c) -> p h c", h=H))

        identh_t = const1.tile([128, 128], F16)
        nc.scalar.copy(out=identh_t[:, :], in_=identh_v)
        identh = identh_t[:, :]
        eps_t = const1.tile([128, 1], F32)
        nc.vector.memset(eps_t[:, :], EPS)
        expb_t = const1.tile([S, 1], F32)
        nc.vector.memset(expb_t[:, :], EXPB)
        ones65B = const1.tile([S, C], F16)
        nc.vector.memset(ones65B[:, :], 1.0)
        rhs5 = const1.tile([5, 1], F32)
        nc.vector.memset(rhs5[:, :], 1.0)
        # preload ACT set 0 (Exp/Square/Copy) once, while DMAs stream
        dummy = stat.tile([1, 1], F32)
        nc.vector.memset(dummy[:, :], 1.0)
        nc.scalar.activation(out=dummy[:, :], in_=dummy[:, :], func=AF.Exp)

        # PE warmups on the eps tile (lift p-state early)
        nc.tensor.matmul(out=PS[0:1, PB_WARM:PB_WARM + 1], lhsT=eps_t[:, :],
                         rhs=eps_t[:, :], start=True, stop=True)
        nc.tensor.matmul(out=PS[0:1, PB_WARM:PB_WARM + 1], lhsT=eps_t[:, :],
                         rhs=eps_t[:, :], start=True, stop=True)
        # pwB broadcast via K=1 matmuls; dots read it from PSUM directly
        ones_t = pwo_sb[0:1, PATCH:PATCH + 128]
        for q4 in range(4):
            nc.tensor.matmul(out=PS[:, PB_PWB + 512 * q4:
                                    PB_PWB + 512 * (q4 + 1)],
                             lhsT=ones_t,
                             rhs=pwo_sb[0:1, 512 * q4:512 * (q4 + 1)],
                             start=True, stop=True)

        # ---------------- phase 1: per-patch stats ----------------
        junk = junkp.tile([128, PATCH], F32, tag="junk")
        junk2 = junkp.tile([128, PATCH], F32, tag="junk2")
        junk3 = junkp.tile([128, PATCH], F32, tag="junk3")
        junk23 = junkp.tile([128, PATCH], F32, tag="junk23")
        ss_c = stat.tile([128, 4], F32, tag="ss_c")
        dot_c = stat.tile([128, 4], F32, tag="dot_c")
        ss3 = stat.tile([128, 4], F32, tag="ss3")
        dot3 = stat.tile([128, 4], F32, tag="dot3")
        msx = stat.tile([128, 4], F32, tag="msx")
        dd = stat.tile([128, 4], F32, tag="dd")
        logit_c = stat.tile([128, 4], F32, tag="logit_c")

        nd = stat.tile([128, 4], F32, tag="nd")
        rms = stat.tile([128, 4], F32, tag="rms")

        def logit_tile(i, eng):
            # ms = ss/PATCH + EPS; logit' = dot*|dot| * recip(ms) (order-eq)
            nc.vector.tensor_scalar(
                out=msx[:, i:i + 1], in0=ss_c[:, i:i + 1],
                scalar1=1.0 / PATCH, scalar2=EPS, op0=ALU.mult, op1=ALU.add)
            nc.vector.reciprocal(out=rms[:, i:i + 1], in_=msx[:, i:i + 1])
            nc.vector.tensor_scalar(
                out=nd[:, i:i + 1].bitcast(I32),
                in0=dot_c[:, i:i + 1].bitcast(I32),
                scalar1=0x7FFFFFFF, scalar2=None, op0=ALU.bitwise_and)
            nc.vector.tensor_tensor(
                out=dd[:, i:i + 1], in0=nd[:, i:i + 1],
                in1=dot_c[:, i:i + 1], op=ALU.mult)
            nc.vector.tensor_tensor(
                out=logit_c[:, i:i + 1], in0=dd[:, i:i + 1],
                in1=rms[:, i:i + 1], op=ALU.mult)
            nc.tensor.transpose(
                out=PS[0:1, LROW + 128 * i:LROW + 128 * (i + 1)],
                in_=logit_c[:, i:i + 1], identity=identf[:, :])

        for i in range(3):
            xp = xps[i]
            nc.scalar.activation(out=junk[:, :], in_=xp[:, :],
                                 func=AF.Square,
                                 accum_out=ss_c[:, i:i + 1])
            nc.vector.scalar_tensor_tensor(
                out=junk2[:, :], in0=xp[:, :], scalar=1.0,
                in1=PS[:, PB_PWB:PB_PWB + PATCH],
                op0=ALU.mult, op1=ALU.mult,
                accum_out=dot_c[:, i:i + 1])
            logit_tile(i, nc.vector)

        # tile 3 chunks: ACT squares a,b,d + Pool square c;
        # Pool dots a,b + DVE dots c,d
        def sq3(eng, ch):
            cs = slice(512 * ch, 512 * (ch + 1))
            if eng is nc.scalar:
                nc.scalar.activation(out=junk3[:, cs], in_=xp3[:, cs],
                                     func=AF.Square,
                                     accum_out=ss3[:, ch:ch + 1])
            else:
                eng.scalar_tensor_tensor(
                    out=junk3[:, cs], in0=xp3[:, cs], scalar=1.0,
                    in1=xp3[:, cs], op0=ALU.mult, op1=ALU.mult,
                    accum_out=ss3[:, ch:ch + 1])

        def dot3f(eng, ch):
            cs = slice(512 * ch, 512 * (ch + 1))
            nc.vector.scalar_tensor_tensor(
                out=junk23[:, cs], in0=xp3[:, cs], scalar=1.0,
                in1=PS[:, PB_PWB + 512 * ch:PB_PWB + 512 * (ch + 1)],
                op0=ALU.mult, op1=ALU.mult,
                accum_out=dot3[:, ch:ch + 1])

        dot3f(nc.vector, 0)
        sq3(nc.scalar, 0)
        dot3f(nc.vector, 1)
        sq3(nc.scalar, 1)
        dot3f(nc.vector, 2)
        sq3(nc.scalar, 2)
        dot3f(nc.vector, 3)
        sq3(nc.scalar, 3)
        nc.vector.tensor_reduce(out=ss_c[:, 3:4],
                                in_=ss3[:, :].rearrange("p (a f) -> p a f",
                                                        a=1),
                                axis=AX.X, op=ALU.add)
        nc.vector.tensor_reduce(out=dot_c[:, 3:4],
                                in_=dot3[:, :].rearrange("p (a f) -> p a f",
                                                         a=1),
                                axis=AX.X, op=ALU.add)
        logit_tile(3, nc.vector)

        # ---------------- top-4 selection (global, on the PSUM row) --------
        lrow = PS[0:1, LROW:LROW + NP]
        gmax8 = stat.tile([1, 8], F32, tag="gmax8")
        nc.vector.max(out=gmax8[:, :], in_=lrow)
        masked = stat.tile([1, NP], F32, tag="masked")
        nc.vector.scalar_tensor_tensor(
            out=masked[:, :], in0=lrow, scalar=gmax8[:, 3:4],
            in1=negio, op0=ALU.is_ge, op1=ALU.mult)
        mm8 = stat.tile([1, 8], F32, tag="mm8")
        nc.vector.max(out=mm8[:, :], in_=masked[:, :])

        # patch ranks (NP - v) fold into sel16'; rhs = [v0..v3, 1]
        nc.tensor.transpose(out=PS[0:4, PB_MM8:PB_MM8 + 1],
                            in_=mm8[0:1, 0:4], identity=identf[0:1, 0:1])
        nc.scalar.copy(out=rhs5[0:4, :], in_=PS[0:4, PB_MM8:PB_MM8 + 1])
        nc.tensor.matmul(out=PS[0:NSEL, PB_IDC:PB_IDC + 1], lhsT=sel16,
                         rhs=rhs5[:, :], start=True, stop=True)
        idc_i = stat.tile([NSEL, 1], I32, tag="idc_i")
        nc.vector.tensor_copy(out=idc_i[:, :],
                              in_=PS[0:NSEL, PB_IDC:PB_IDC + 1])

        if LEVEL == 1:
            l1 = stat.tile([NSEL, C], F32, tag="l1")
            nc.vector.tensor_copy(out=l1[0:4, 0:8],
                                  in_=mm8[0:1, :].to_broadcast([4, 8]))
            nc.sync.dma_start(out=out[:, :], in_=l1[:, :])
            return

        # gather the 64 tokens (row 16T+4p+t = token 16*I[p] + 4T + t)
        x_sel = sb.tile([NSEL, C], F32, tag="x_sel")
        nc.gpsimd.indirect_dma_start(
            out=x_sel[:, :], out_offset=None, in_=xb[:, :],
            in_offset=bass.IndirectOffsetOnAxis(ap=idc_i[:, 0:1], axis=0))

        if LEVEL == 2:
            nc.sync.dma_start(out=out[:, :], in_=x_sel[:, :])
            return

        # ---------------- qkvg projection (fp16) ----------------
        nc.tensor.transpose(out=PS[0:128, PB_XSELT:PB_XSELT + NSEL],
                            in_=x_sel[:, :],
                            identity=identf[0:NSEL, 0:NSEL])
        x_selT = sb.tile([C, NSEL], F16, tag="x_selT")
        nc.scalar.copy(out=x_selT[:, :], in_=PS[:, PB_XSELT:PB_XSELT + NSEL])

        for g in range(8):
            nc.tensor.matmul(out=PS[0:NSEL, 512 * g:512 * (g + 1)],
                             lhsT=x_selT[:, :],
                             rhs=wqT[:, 512 * g:512 * (g + 1)],
                             start=True, stop=True)

        # staging to fp16: qk rows 0:32 by block (ACT/DVE/Pool/ACT),
        # then vg rows 32:64
        stQK = sb.tile([32, 4 * C * H], F16, tag="stQK")
        stVG = sb.tile([32, 4 * C * H], F16, tag="stVG")
        nc.scalar.copy(out=stQK[:, 0:1024], in_=PS[0:32, 0:1024])
        nc.vector.tensor_copy(out=stQK[:, 1024:2048], in_=PS[0:32, 1024:2048])
        nc.scalar.copy(out=stQK[:, 2048:3072], in_=PS[0:32, 2048:3072])
        nc.vector.tensor_copy(out=stQK[:, 3072:4096], in_=PS[0:32, 3072:4096])
        # qk rearrange: src iterates (r, b, col) matching plain dst
        # partition order 4r+b exactly
        qk = sb.tile([128, H, C], F16, tag="qk")
        FQ = 4 * C * H
        nc.sync.dma_start(
            out=qk[:, :, :],
            in_=rap(stQK[:, :], [[FQ, 32], [1024, 4], [1, 1024]]))
        nc.scalar.copy(out=stVG[:, 0:1024], in_=PS[32:64, 0:1024])
        nc.vector.tensor_copy(out=stVG[:, 1024:2048], in_=PS[32:64, 1024:2048])
        nc.scalar.copy(out=stVG[:, 2048:3072], in_=PS[32:64, 2048:3072])
        nc.vector.tensor_copy(out=stVG[:, 3072:4096], in_=PS[32:64, 3072:4096])

        # g rearrange (rows 16:32 of stVG), then v (rows 0:16) into v_sb
        g_sb = sb.tile([NSEL, H, C], F16, tag="g_sb")
        nc.sync.dma_start(
            out=g_sb[:, :, :],
            in_=rap(stVG[:, :], [[FQ, 16], [1024, 4], [1, 1024]],
                    offset=16 * FQ))
        nc.sync.dma_start(
            out=v_sb[0:NSEL, :, :],
            in_=rap(stVG[:, :], [[FQ, 16], [1024, 4], [1, 1024]]))

        if LEVEL == 3:
            l3 = sb.tile([NSEL, C], F32, tag="l3")
            nc.vector.tensor_copy(out=l3[:, :], in_=qk[0:NSEL, 0, :])
            nc.sync.dma_start(out=out[:, :], in_=l3[:, :])
            return

        # causal-mask preload for both att groups (PE idle window)
        for g in range(2):
            attb = PB_ATT0 if g == 0 else PB_ATT1
            nc.tensor.matmul(
                out=PS[0:S, attb:attb + 4 * S],
                lhsT=identh[0:S, 0:S],
                rhs=rap(t2h, [[2 * TB_COLS, 65], [0, 4], [1, 65]],
                        offset=2 * TB_CMASK),
                start=True, stop=False)

        # ---------------- rmsnorm + rope (fp16) ----------------
        # squares: ACT heads 5:8 (accum), DVE heads 0:5 (fp16 2x + reduce)
        ssq = sb.tile([128, H], F32, tag="ssq")
        sqj = junkp.tile([128, 5, C], F16, tag="sqj")
        sqa = junkp.tile([128, 3, C], F32, tag="sqa")
        nc.vector.tensor_tensor(out=sqj[:, :, :], in0=qk[:, 0:5, :],
                                in1=qk[:, 0:5, :], op=ALU.mult)
        nc.vector.tensor_reduce(out=ssq[:, 0:5], in_=sqj[:, :, :],
                                axis=AX.X, op=ALU.add)
        for h in range(5, 8):
            nc.scalar.activation(out=sqa[:, h - 5, :], in_=qk[:, h, :],
                                 func=AF.Square,
                                 accum_out=ssq[:, h:h + 1])
        # rope (independent of rf): r1 = qk*cos; r2 = swap(qk)*sin
        r1 = sb.tile([128, H, C], F16, tag="r1")
        r2 = sb.tile([128, H, C], F16, tag="r2")
        qkr = sb.tile([128, H, C], F16, tag="qkr")
        qkn = sb.tile([128, H, C], F16, tag="qkn")
        nc.vector.tensor_tensor(
            out=r1[:, :, :], in0=qk[:, :, :],
            in1=cosdup.rearrange("p (a c) -> p a c", a=1)
                .to_broadcast([128, H, C]), op=ALU.mult)
        nc.vector.tensor_tensor(
            out=r2[:, :, 0:64], in0=qk[:, :, 64:128],
            in1=rap(t2h, [[2 * TB_COLS, 128], [0, H], [1, 64]],
                    offset=2 * TB_SIN),
            op=ALU.mult)
        nc.vector.tensor_tensor(
            out=r2[:, :, 64:128], in0=qk[:, :, 0:64],
            in1=rap(t2h, [[2 * TB_COLS, 128], [0, H], [1, 64]],
                    offset=2 * TB_SIN + 64),
            op=ALU.mult)
        nc.vector.tensor_add(out=qkr[:, :, :], in0=r1[:, :, :],
                             in1=r2[:, :, :])
        # rf = rsqrt(ssq/C + eps) via bit-trick + 2 Newton steps, on Pool
        msv = sb.tile([128, H], F32, tag="msv")
        nwa = sb.tile([128, H], F32, tag="nwa")
        nwb = sb.tile([128, H], F32, tag="nwb")
        yv = sb.tile([128, H], F32, tag="yv")
        rf = sb.tile([128, H], F16, tag="rf")
        nc.gpsimd.tensor_scalar(out=msv[:, :], in0=ssq[:, :],
                                scalar1=1.0 / C, scalar2=EPS,
                                op0=ALU.mult, op1=ALU.add)
        msv_i = msv[:, :].bitcast(I32)
        yv_i = yv[:, :].bitcast(I32)
        nc.vector.tensor_scalar(out=yv_i, in0=msv_i, scalar1=1,
                                scalar2=None, op0=ALU.arith_shift_right)
        nc.vector.tensor_scalar(out=yv_i, in0=yv_i, scalar1=-1,
                                scalar2=MAGIC, op0=ALU.mult, op1=ALU.add)
        for it in range(2):
            eng = nc.gpsimd if it == 0 else nc.vector
            eng.tensor_tensor(out=nwa[:, :], in0=yv[:, :],
                              in1=yv[:, :], op=ALU.mult)
            eng.tensor_tensor(out=nwb[:, :], in0=nwa[:, :],
                              in1=msv[:, :], op=ALU.mult)
            eng.tensor_scalar(out=nwb[:, :], in0=nwb[:, :],
                              scalar1=-0.5, scalar2=1.5,
                              op0=ALU.mult, op1=ALU.add)
            eng.tensor_tensor(out=yv[:, :], in0=yv[:, :],
                              in1=nwb[:, :], op=ALU.mult)
        nc.vector.tensor_copy(out=rf[:, :], in_=yv[:, :])
        # qkn = qkr * rf (broadcast over c): g0 on DVE first, then g1
        nc.vector.tensor_tensor(
            out=qkn[:, 0:4, :], in0=qkr[:, 0:4, :],
            in1=rf[:, 0:4].rearrange("p (h a) -> p h a", a=1)
                .to_broadcast([128, 4, C]), op=ALU.mult)
        nc.vector.tensor_tensor(
            out=qkn[:, 4:6, :], in0=qkr[:, 4:6, :],
            in1=rf[:, 4:6].rearrange("p (h a) -> p h a", a=1)
                .to_broadcast([128, 2, C]), op=ALU.mult)
        nc.gpsimd.tensor_tensor(
            out=qkn[:, 6:8, :], in0=qkr[:, 6:8, :],
            in1=rf[:, 6:8].rearrange("p (h a) -> p h a", a=1)
                .to_broadcast([128, 2, C]), op=ALU.mult)

        if LEVEL == 4:
            l4 = sb.tile([NSEL, C], F32, tag="l4")
            nc.vector.tensor_copy(out=l4[:, :], in_=qkn[0:NSEL, 0, :])
            nc.sync.dma_start(out=out[:, :], in_=l4[:, :])
            return

        # ---------------- transposes to qnT / knT (fp16 via PSUM bitcast) --
        qnT = sb.tile([C, H, S], F16, tag="qnT")
        knT = sb.tile([C, H, S], F16, tag="knT")
        nc.scalar.copy(out=rap(qnT[:, :, :], [[H * S, C], [S, H], [1, 1]],
                               offset=NSEL),
                       in_=sinkTq.rearrange("c (h a) -> c h a", a=1))
        nc.scalar.copy(out=rap(knT[:, :, :], [[H * S, C], [S, H], [1, 1]],
                               offset=NSEL),
                       in_=sinkTk.rearrange("c (h a) -> c h a", a=1))

        if LEVEL == 41:
            l41 = sb.tile([NSEL, C], F32, tag="l41")
            nc.vector.tensor_copy(out=l41[:, 0:8],
                                  in_=qnT[0:NSEL, 0, 0:8])
            nc.vector.memset(l41[:, 8:128], 0.0)
            nc.sync.dma_start(out=out[:, :], in_=l41[:, :])
            return

        pexp = sb.tile([S, H, S], F16, tag="pexp")
        for g in range(2):
            base16 = 512 * g
            # one [128,128] transpose per head: out cols 0:64 = q^T,
            # 64:128 = k^T (tile_position (0,0); fp16 (64,0) faults)
            for j in range(4):
                h = 4 * g + j
                nc.tensor.transpose(
                    out=rap(PS16, [[8192, 128], [1, 128]],
                            offset=base16 + 128 * j),
                    in_=qkn[:, h, :],
                    identity=identh)
            for si, dstT in enumerate((qnT, knT)):
                dst = rap(dstT[:, :, :], [[H * S, C], [S, 4], [1, NSEL]],
                          offset=4 * g * S)
                src = rap(PS16, [[8192, 128], [128, 4], [1, NSEL]],
                          offset=base16 + 64 * si)
                nc.vector.tensor_copy(out=dst, in_=src)
            if LEVEL == 40 + 5 * g + 3:  # 43->g0, 48->g1
                l43 = sb.tile([NSEL, C], F32, tag="l43")
                nc.vector.memset(l43[:, :], 0.0)
                nc.vector.tensor_copy(out=l43[:, 0:65],
                                      in_=qnT[0:NSEL, 4 * g, :])
                nc.sync.dma_start(out=out[:, :], in_=l43[:, :])
                return
            # attention: QK^T accumulated onto the preloaded mask, then exp
            attb = PB_ATT0 if g == 0 else PB_ATT1
            for j in range(4):
                h = 4 * g + j
                nc.tensor.matmul(out=PS[0:S, attb + S * j:attb + S * (j + 1)],
                                 lhsT=knT[:, h, :], rhs=qnT[:, h, :],
                                 start=False, stop=(j == 3))
            nc.scalar.activation(
                out=pexp[:, 4 * g:4 * (g + 1), :],
                in_=PS[0:S, attb:attb + 4 * S].rearrange(
                    "p (h s) -> p h s", h=4),
                func=AF.Exp, bias=expb_t[:, :], scale=SCALE)
            if LEVEL == 42 + g:
                l42 = sb.tile([NSEL, C], F32, tag="l42")
                nc.vector.memset(l42[:, :], 0.0)
                nc.vector.tensor_copy(out=l42[:, 0:64],
                                      in_=pexp[0:NSEL, 4 * g, 0:64])
                nc.sync.dma_start(out=out[:, :], in_=l42[:, :])
                return
        if LEVEL == 5:
            l5 = sb.tile([NSEL, C], F32, tag="l5")
            nc.vector.tensor_copy(out=l5[:, 0:64], in_=pexp[0:NSEL, 0, 0:64])
            nc.vector.tensor_copy(out=l5[:, 64:128], in_=pexp[0:NSEL, 1, 0:64])
            nc.sync.dma_start(out=out[:, :], in_=l5[:, :])
            return

        # gT transposes + e_gT = exp(-g) (fp16)
        e_gT = sb.tile([C, H, NSEL], F16, tag="e_gT")
        for h in range(H):
            nc.tensor.transpose(
                out=rap(PS16, [[8192, 128], [1, NSEL]],
                        offset=PB_GT16 + NSEL * h),
                in_=g_sb[:, h, :],
                identity=identh[0:NSEL, 0:NSEL])
        nc.scalar.activation(
            out=e_gT[:, :, :],
            in_=rap(PS16, [[8192, 128], [NSEL, H], [1, NSEL]],
                    offset=PB_GT16).rearrange("p h s -> p h s"),
            func=AF.Exp, scale=-1.0)

        # den broadcast to all 128 partitions via ones[65,128] matmul
        nc.tensor.matmul(
            out=PS[0:128, PB_DENB:PB_DENB + 512],
            lhsT=ones65B[:, :],
            rhs=rap(pexp[:, :, :], [[H * S, S], [S, H], [1, NSEL]]),
            start=True, stop=True)

        # yT per head (unnormalized p), then yg^T = yT / ((1+e^-g)*den)
        for h in range(H):
            nc.tensor.matmul(out=PS[0:C, PB_YT + NSEL * h:
                                     PB_YT + NSEL * (h + 1)],
                             lhsT=v_sb[:, h, :], rhs=pexp[:, h, 0:NSEL],
                             start=True, stop=True)
        D = sb.tile([128, 512], F32, tag="D")
        ygT = sb.tile([C, H, NSEL], F16, tag="ygT")
        egT_flat = e_gT[:, :, :].rearrange("p h s -> p (h s)")
        ygT_flat = ygT[:, :, :].rearrange("p h s -> p (h s)")
        Dr = sb.tile([128, 512], F32, tag="Dr")
        nc.vector.scalar_tensor_tensor(
            out=D[:, 0:256], in0=egT_flat[:, 0:256],
            scalar=1.0, in1=PS[:, PB_DENB:PB_DENB + 256],
            op0=ALU.add, op1=ALU.mult)
        nc.vector.reciprocal(out=Dr[:, 0:256], in_=D[:, 0:256])
        nc.vector.tensor_tensor(
            out=ygT_flat[:, 0:256],
            in0=PS[:, PB_YT:PB_YT + 256], in1=Dr[:, 0:256], op=ALU.mult)
        nc.vector.scalar_tensor_tensor(
            out=D[:, 256:512], in0=egT_flat[:, 256:512],
            scalar=1.0, in1=PS[:, PB_DENB + 256:PB_DENB + 512],
            op0=ALU.add, op1=ALU.mult)
        nc.vector.reciprocal(out=Dr[:, 256:512], in_=D[:, 256:512])
        nc.vector.tensor_tensor(
            out=ygT_flat[:, 256:512],
            in0=PS[:, PB_YT + 256:PB_YT + 512], in1=Dr[:, 256:512],
            op=ALU.mult)

        # ---------------- output projection ----------------
        out_ps = PS[0:NSEL, PB_OUT:PB_OUT + C]
        out_sb = sb.tile([NSEL, C], F32, tag="out_sb")
        for h in range(H):
            nc.tensor.matmul(out=out_ps, lhsT=ygT[:, h, :],
                             rhs=woTb[:, h, :], start=(h == 0),
                             stop=(h == H - 1))
        nc.vector.tensor_copy(out=out_sb[:, :], in_=out_ps)
        nc.sync.dma_start(out=out[:, :], in_=out_sb[:, :])


def make_host_constants(inputs):
    """Host-side prep of tables derived from the (full) inputs."""
    cos = np.asarray(inputs["cos"]).reshape(S, 64).astype(np.float32)
    sin = np.asarray(inputs["sin"]).reshape(S, 64).astype(np.float32)
    sink = np.asarray(inputs["sink"]).reshape(H, C).astype(np.float32)
    tao = np.asarray(inputs["tao"]).astype(np.float32)
    wq = np.asarray(inputs["W_qkvg"]).astype(np.float32)
    wo = np.asarray(inputs["W_out"]).astype(np.float32)

    pos = np.arange(64) + 1
    cos_p = cos[pos]
    sin_p = sin[pos]
    cosdup = np.tile(np.concatenate([cos_p, cos_p], axis=1), (2, 1))
    sinpm = np.tile(np.concatenate([sin_p, -sin_p], axis=1), (2, 1))
    taocol = np.concatenate([np.full((64, 1), tao[0], np.float32),
                             np.full((64, 1), tao[1], np.float32)])
    cosdup = (cosdup * taocol).astype(np.float16)
    sinpm = (sinpm * taocol).astype(np.float16)

    # additive causal mask TRANSPOSED: mask[t, s] = 0 if pos_t <= pos_s
    posf = np.where(np.arange(S) < NSEL, np.arange(S) + 1, 0)
    cmaskm = np.where(posf[:, None] <= posf[None, :], 0.0,
                      NEG_BIG).astype(np.float16)  # [t, s]

    sn = sink / np.sqrt((sink * sink).mean(axis=-1, keepdims=True) + EPS)
    sinkTq = np.ascontiguousarray((sn * tao[0]).T).astype(np.float16)
    sinkTk = np.ascontiguousarray((sn * tao[1]).T).astype(np.float16)
    sinkv = sink.reshape(1, H * C).astype(np.float16)

    # sel16'[p, r] = -16 if p == p(r); sel16'[4, r] = 16*NP + 4*T(r) + t(r)
    sel16m = np.zeros((5, NSEL), np.float32)
    for Tn in range(4):
        for p in range(4):
            for t in range(4):
                r = 16 * Tn + 4 * p + t
                sel16m[p, r] = -16.0
                sel16m[4, r] = float(16 * NP + 4 * Tn + t)

    negio = (float(NP) - np.arange(NP, dtype=np.float32)).astype(np.float16)

    wqT = np.ascontiguousarray(wq.T).astype(np.float16)
    woT = np.ascontiguousarray(
        wo.reshape(C, H, C).transpose(2, 1, 0)).astype(np.float16)

    identf32 = np.eye(128, dtype=np.float32)
    identf16 = np.eye(128, dtype=np.float16)

    def pack16(a, rows=128):
        c = a.shape[1]
        padded = np.zeros((128, c), np.float16)
        padded[:rows] = a
        return padded.view(np.float32)

    tabs2 = np.zeros((128, TB_COLS), np.float32)
    tabs2[0:5, TB_SEL16:TB_SEL16 + 64] = sel16m
    tabs2[:, TB_IDF16:TB_IDF16 + 64] = pack16(identf16)
    tabs2[:, TB_COS:TB_COS + 64] = pack16(cosdup)
    tabs2[:, TB_SIN:TB_SIN + 64] = pack16(sinpm)
    tabs2[:, TB_SINKQ:TB_SINKQ + 4] = pack16(sinkTq)
    tabs2[:, TB_SINKK:TB_SINKK + 4] = pack16(sinkTk)
    cm = np.zeros((65, 66), np.float16)
    cm[:, 0:65] = cmaskm
    tabs2[:, TB_CMASK:TB_CMASK + 33] = pack16(cm, rows=65)
    tabs2[:, TB_NEGIO:TB_NEGIO + 256] = pack16(negio.reshape(1, NP), rows=1)
    return dict(tabs2=tabs2, identd=identf32, sinkv=sinkv,
                wqT_d=wqT, woT_d=woT)


_CACHE = {}


def get_nc():
    if "nc" not in _CACHE:
        nc = bacc.Bacc("TRN2", target_bir_lowering=False, debug=False,
                       num_devices=B)
        build_kernel(nc)
        nc.compile()
        _CACHE["nc"] = nc
    return _CACHE["nc"]


def make_in_maps(inputs):
    x = np.ascontiguousarray(inputs["x"], dtype=np.float32)
    pwv = np.concatenate(
        [np.asarray(inputs["patch_w"], np.float32).ravel(),
         np.ones(128, np.float32)]).reshape(1, PATCH + 128)
    consts = make_host_constants(inputs)
    in_maps = []
    for b in range(B):
        m = {"xb": np.ascontiguousarray(x[b]), "pw": pwv}
        m.update(consts)
        in_maps.append(m)
    return in_maps


def kernel(**inputs):
    nc = get_nc()
    in_maps = make_in_maps(inputs)
    res = run_bass_kernel_spmd(nc, in_maps, core_ids=list(range(B)))
    return np.stack([r["out"] for r in res.results], axis=0)


if __name__ == "__main__":
    nc = get_nc()
    print("build ok:", len(nc.m.functions[0].allocations), "allocations")


# revision 34
# speedup vs baseline: 1.0611x; 1.0021x over previous
"""Trainium2 Bass kernel for nn_AttentionOnDetail (sparse patch attention).

Data-parallel over batch B=8 across 8 NeuronCores; one batch per core.
v3 redesign (latency-focused; the kernel is dependency-bound):
  - x streamed first (x tile DMAs are the first SP descriptors; tile 3 in
    four 512-col chunks).  pw row + f32 ident ride gpsimd SWDGE so their
    transfers slip into the stream right after tile 0.
  - pw broadcast stays in PSUM (dot stt reads PSUM directly, no copies);
    PE warmup matmuls run on the eps tile at t~0.9 so everything after
    runs at peak p-state.
  - stats split across engines: ACT squares (t0-t2, 3a, 3b, 3d), DVE dots
    (t0-t2, 3c, 3d) + logit chain, Pool dots (3a, 3b) + square (3c).
  - patch logits use the monotone transform dot*|dot|/ms (no Sqrt); the
    selection runs once globally: top8 -> threshold mask * negio -> top8
    gives the ranks directly (idc matmul folds NP-v and the *16).
  - single ACT function set (Exp/Square/Copy, set 0) loaded once at t=0;
    rmsnorm rsqrt = Newton iteration on Pool (bit-trick seed), sigmoid
    via exp(-g), softmax exp with folded -6 bias so p fits fp16.
  - everything from the projection on runs in fp16 (W cast on host).
  - attention computed transposed (att_T = k^T q) with the causal mask
    preloaded into PSUM via an identity matmul; denominator broadcast to
    all partitions by a single ones[65,128] matmul; gate folded as
    yg = y / ((1+e^-g)*den) with DVE/Pool divide; output projection
    consumes yg^T directly.
"""

import sys
import numpy as np

for _p in ("/opt/trn_rl_repo",):
    if _p not in sys.path:
        sys.path.insert(0, _p)

import concourse.bass as bass
import concourse.bacc as bacc
import concourse.tile as tile
from concourse import mybir
from concourse.bass_utils import run_bass_kernel_spmd

F32 = mybir.dt.float32
F32R = mybir.dt.float32r
F16 = mybir.dt.float16
I32 = mybir.dt.int32
U32 = mybir.dt.uint32
AF = mybir.ActivationFunctionType
ALU = mybir.AluOpType
AX = mybir.AxisListType

B, T, C, H, T0 = 8, 8192, 128, 8, 16
NP = T // T0          # 512 patches
PATCH = T0 * C        # 2048 elements per patch
S = 65                # sink + 64 selected tokens
NSEL = 64
EPS = 1.1920929e-07
SCALE = 1.0 / float(np.sqrt(np.float32(C)))
EXPB = -6.0           # softmax exp bias; den-normalization cancels it
NEG_BIG = -60000.0    # additive causal mask (fp16-representable)
MAGIC = 0x5F3759DF    # fast-rsqrt seed

# tabs2 f32-column layout (fp16 payloads packed as pairs into f32 cols)
TB_SEL16 = 0          # sel16' f32 [5, 64]
TB_IDF16 = 64         # ident f16 [128, 128] -> 64 f32 cols
TB_COS = 128          # cosdup f16 [128, 128] -> 64
TB_SIN = 192          # sinpm f16 [128, 128] -> 64
TB_SINKQ = 256        # sinkTq f16 [128, 8] -> 4
TB_SINKK = 260        # sinkTk f16 [128, 8] -> 4
TB_CMASK = 264        # cmaskT f16 [65, 66] -> 33 (col 65 pad)
TB_NEGIO = 297        # negio f16 [1, 512] -> 256
TB_COLS = 553

# PSUM f32-column region plan (8 banks x 512 cols)
PB_PWB = 0            # pwB broadcast [128, 2048] (cols 0:2048), early only
PB_QNT16 = 0          # qkn transposes (f16 cols 0:1024 = f32 0:512)
PB_GT16 = 2048        # gT transposes (f16 cols 2048:2560 = f32 1024:1280)
PB_YT = 1536          # yT [128, (h,s)=512] cols 1536:2048 (bank 3)
PB_OUT = 3584         # out [64, 128] in bank 7 (logits row dead)
PB_ATT0 = 2048        # att_T group 0 [65, 260]
PB_ATT1 = 2560        # att_T group 1 [65, 260]
PB_XSELT = 3072       # x_selT staging [128, 64]
PB_DENB = 3072        # den broadcast [128, 512] (after x_selT dead)
PB_WARM = 3500        # warmup scratch col
PB_MM8 = 3582         # mm8 transpose column [4, 1]
PB_IDC = 3583         # sel16 matmul out [64, 1]
LROW = 3584           # logits row [1, 512] (rows 0:1)


def rap(t, apl, offset=0):
    """Raw AP over a tile/AP's storage, flat element strides."""
    base = t if isinstance(t, bass.AP) else t[:]
    return bass.AP(tensor=base.tensor, offset=base.offset + offset,
                   ap=[list(x) for x in apl])


def build_kernel(nc):
    xb = nc.dram_tensor("xb", [T, C], F32, kind="ExternalInput")
    pw = nc.dram_tensor("pw", [1, PATCH + 128], F32R, kind="ExternalInput")
    identd = nc.dram_tensor("identd", [128, 128], F32, kind="ExternalInput")
    tabs2 = nc.dram_tensor("tabs2", [128, TB_COLS], F32, kind="ExternalInput")
    wqT_d = nc.dram_tensor("wqT_d", [C, 4 * C * H], F16, kind="ExternalInput")
    woT_d = nc.dram_tensor("woT_d", [C, H, C], F16, kind="ExternalInput")
    sinkv = nc.dram_tensor("sinkv", [1, H * C], F16, kind="ExternalInput")
    out = nc.dram_tensor("out", [NSEL, C], F32, kind="ExternalOutput")

    with tile.TileContext(nc) as tc:
        _emit(tc, nc, xb, pw, identd, tabs2, wqT_d, woT_d, sinkv, out)
    return nc


def _emit(tc, nc, xb, pw, identd, tabs2_d, wqT_d, woT_d, sinkv, out):
    import os
    LEVEL = int(os.environ.get("KLEVEL", "9"))
    from contextlib import ExitStack
    ctx = ExitStack()
    with ctx:
        const1 = ctx.enter_context(tc.tile_pool(name="const1", bufs=1))
        xpool = ctx.enter_context(tc.tile_pool(name="xpool", bufs=1))
        junkp = ctx.enter_context(tc.tile_pool(name="junkp", bufs=1))
        stat = ctx.enter_context(tc.tile_pool(name="stat", bufs=4))
        sb = ctx.enter_context(tc.tile_pool(name="sb", bufs=1))
        psall = ctx.enter_context(tc.tile_pool(name="psall", bufs=1,
                                               space="PSUM"))
        PS = psall.tile([128, 4096], F32)
        PS16 = PS[:, :].bitcast(F16)  # [128, 8192] f16 view

        # ---------------- x stream first; pw/ident via gpsimd --------------
        def xdma(i):
            xp = xpool.tile([128, PATCH], F32, tag=f"xp{i}")
            nc.sync.dma_start(
                out=xp[:, :],
                in_=rap(xb[:, :], [[PATCH, 128], [1, PATCH]],
                        offset=i * 128 * PATCH))
            return xp

        xps = [xdma(0)]
        pwo_sb = const1.tile([1, PATCH + 128], F32R)
        nc.gpsimd.dma_start(out=pwo_sb[:, :], in_=pw[:, :])
        identf = const1.tile([128, 128], F32)
        nc.gpsimd.dma_start(out=identf[:, :], in_=identd[:, :])
        xps.append(xdma(1))
        xps.append(xdma(2))
        xp3 = xpool.tile([128, PATCH], F32, tag="xp3")
        for ch in range(4):
            nc.sync.dma_start(
                out=xp3[:, 512 * ch:512 * (ch + 1)],
                in_=rap(xb[:, :], [[PATCH, 128], [1, 512]],
                        offset=3 * 128 * PATCH + 512 * ch))
        xps.append(xp3)

        tabs2 = const1.tile([128, TB_COLS], F32)
        nc.sync.dma_start(out=tabs2[:, :], in_=tabs2_d[:, :])
        t2h = tabs2[:, :].bitcast(F16)  # [128, 2*TB_COLS] f16 view

        def h16(col_f32, ncols_f16, nrows=128):
            return rap(t2h, [[2 * TB_COLS, nrows], [1, ncols_f16]],
                       offset=2 * col_f32)

        identh_v = h16(TB_IDF16, 128)
        identh = None  # materialized below after tabs2 lands
        cosdup = h16(TB_COS, 128)
        sinkTq = h16(TB_SINKQ, 8)
        sinkTk = h16(TB_SINKK, 8)
        negio = h16(TB_NEGIO, 512, nrows=1)
        sel16 = tabs2[0:5, TB_SEL16:TB_SEL16 + 64]

        wqT = const1.tile([C, 4 * C * H], F16)
        for wch in range(4):
            nc.sync.dma_start(out=wqT[:, 1024 * wch:1024 * (wch + 1)],
                              in_=wqT_d[:, 1024 * wch:1024 * (wch + 1)])
        woTb = const1.tile([C, H, C], F16)
        nc.sync.dma_start(out=woTb[:, :, :], in_=woT_d[:, :, :])
        v_sb = sb.tile([S, H, C], F16, tag="v_sb")
        nc.sync.dma_start(
            out=v_sb[NSEL:S, :, :],
            in_=sinkv[:, :].rearrange("p (h c) -> p h c", h=H))

        identh_t = const1.tile([128, 128], F16)
        nc.scalar.copy(out=identh_t[:, :], in_=identh_v)
        identh = identh_t[:, :]
        eps_t = const1.tile([128, 1], F32)
        nc.vector.memset(eps_t[:, :], EPS)
        expb_t = const1.tile([S, 1], F32)
        nc.vector.memset(expb_t[:, :], EXPB)
        ones65B = const1.tile([S, C], F16)
        nc.vector.memset(ones65B[:, :], 1.0)
        rhs5 = const1.tile([5, 1], F32)
        nc.vector.memset(rhs5[:, :], 1.0)
        # preload ACT set 0 (Exp/Square/Copy) once, while DMAs stream
        dummy = stat.tile([1, 1], F32)
        nc.vector.memset(dummy[:, :], 1.0)
        nc.scalar.activation(out=dummy[:, :], in_=dummy[:, :], func=AF.Exp)

        # PE warmups on the eps tile (lift p-state early)
        nc.tensor.matmul(out=PS[0:1, PB_WARM:PB_WARM + 1], lhsT=eps_t[:, :],
                         rhs=eps_t[:, :], start=True, stop=True)
        nc.tensor.matmul(out=PS[0:1, PB_WARM:PB_WARM + 1], lhsT=eps_t[:, :],
                         rhs=eps_t[:, :], start=True, stop=True)
        # pwB broadcast via K=1 matmuls; dots read it from PSUM directly
        ones_t = pwo_sb[0:1, PATCH:PATCH + 128]
        for q4 in range(4):
            nc.tensor.matmul(out=PS[:, PB_PWB + 512 * q4:
                                    PB_PWB + 512 * (q4 + 1)],
                             lhsT=ones_t,
                             rhs=pwo_sb[0:1, 512 * q4:512 * (q4 + 1)],
                             start=True, stop=True)

        # ---------------- phase 1: per-patch stats ----------------
        junk = junkp.tile([128, PATCH], F32, tag="junk")
        junk2 = junkp.tile([128, PATCH], F32, tag="junk2")
        junk3 = junkp.tile([128, PATCH], F32, tag="junk3")
        junk23 = junkp.tile([128, PATCH], F32, tag="junk23")
        ss_c = stat.tile([128, 4], F32, tag="ss_c")
        dot_c = stat.tile([128, 4], F32, tag="dot_c")
        ss3 = stat.tile([128, 4], F32, tag="ss3")
        dot3 = stat.tile([128, 4], F32, tag="dot3")
        msx = stat.tile([128, 4], F32, tag="msx")
        dd = stat.tile([128, 4], F32, tag="dd")
        logit_c = stat.tile([128, 4], F32, tag="logit_c")

        nd = stat.tile([128, 4], F32, tag="nd")
        rms = stat.tile([128, 4], F32, tag="rms")

        def logit_tile(i, eng):
            # ms = ss/PATCH + EPS; logit' = dot*|dot| * recip(ms) (order-eq)
            eng.tensor_scalar(
                out=msx[:, i:i + 1], in0=ss_c[:, i:i + 1],
                scalar1=1.0 / PATCH, scalar2=EPS, op0=ALU.mult, op1=ALU.add)
            nc.vector.reciprocal(out=rms[:, i:i + 1], in_=msx[:, i:i + 1])
            nc.vector.tensor_scalar(
                out=nd[:, i:i + 1].bitcast(I32),
                in0=dot_c[:, i:i + 1].bitcast(I32),
                scalar1=0x7FFFFFFF, scalar2=None, op0=ALU.bitwise_and)
            eng.tensor_tensor(
                out=dd[:, i:i + 1], in0=nd[:, i:i + 1],
                in1=dot_c[:, i:i + 1], op=ALU.mult)
            eng.tensor_tensor(
                out=logit_c[:, i:i + 1], in0=dd[:, i:i + 1],
                in1=rms[:, i:i + 1], op=ALU.mult)
            nc.tensor.transpose(
                out=PS[0:1, LROW + 128 * i:LROW + 128 * (i + 1)],
                in_=logit_c[:, i:i + 1], identity=identf[:, :])

        for i in range(3):
            xp = xps[i]
            nc.scalar.activation(out=junk[:, :], in_=xp[:, :],
                                 func=AF.Square,
                                 accum_out=ss_c[:, i:i + 1])
            nc.vector.scalar_tensor_tensor(
                out=junk2[:, :], in0=xp[:, :], scalar=1.0,
                in1=PS[:, PB_PWB:PB_PWB + PATCH],
                op0=ALU.mult, op1=ALU.mult,
                accum_out=dot_c[:, i:i + 1])
            logit_tile(i, nc.gpsimd)

        # tile 3 chunks: ACT squares a,b,d + Pool square c;
        # Pool dots a,b + DVE dots c,d
        def sq3(eng, ch):
            cs = slice(512 * ch, 512 * (ch + 1))
            if eng is nc.scalar:
                nc.scalar.activation(out=junk3[:, cs], in_=xp3[:, cs],
                                     func=AF.Square,
                                     accum_out=ss3[:, ch:ch + 1])
            else:
                eng.scalar_tensor_tensor(
                    out=junk3[:, cs], in0=xp3[:, cs], scalar=1.0,
                    in1=xp3[:, cs], op0=ALU.mult, op1=ALU.mult,
                    accum_out=ss3[:, ch:ch + 1])

        def dot3f(eng, ch):
            cs = slice(512 * ch, 512 * (ch + 1))
            nc.vector.scalar_tensor_tensor(
                out=junk23[:, cs], in0=xp3[:, cs], scalar=1.0,
                in1=PS[:, PB_PWB + 512 * ch:PB_PWB + 512 * (ch + 1)],
                op0=ALU.mult, op1=ALU.mult,
                accum_out=dot3[:, ch:ch + 1])

        dot3f(nc.vector, 0)
        sq3(nc.scalar, 0)
        dot3f(nc.vector, 1)
        sq3(nc.scalar, 1)
        dot3f(nc.vector, 2)
        sq3(nc.scalar, 2)
        dot3f(nc.vector, 3)
        sq3(nc.scalar, 3)
        nc.vector.tensor_reduce(out=ss_c[:, 3:4],
                                in_=ss3[:, :].rearrange("p (a f) -> p a f",
                                                        a=1),
                                axis=AX.X, op=ALU.add)
        nc.vector.tensor_reduce(out=dot_c[:, 3:4],
                                in_=dot3[:, :].rearrange("p (a f) -> p a f",
                                                         a=1),
                                axis=AX.X, op=ALU.add)
        logit_tile(3, nc.vector)

        # ---------------- top-4 selection (global, on the PSUM row) --------
        lrow = PS[0:1, LROW:LROW + NP]
        gmax8 = stat.tile([1, 8], F32, tag="gmax8")
        nc.vector.max(out=gmax8[:, :], in_=lrow)
        masked = stat.tile([1, NP], F32, tag="masked")
        nc.vector.scalar_tensor_tensor(
            out=masked[:, :], in0=lrow, scalar=gmax8[:, 3:4],
            in1=negio, op0=ALU.is_ge, op1=ALU.mult)
        mm8 = stat.tile([1, 8], F32, tag="mm8")
        nc.vector.max(out=mm8[:, :], in_=masked[:, :])

        # patch ranks (NP - v) fold into sel16'; rhs = [v0..v3, 1]
        nc.tensor.transpose(out=PS[0:4, PB_MM8:PB_MM8 + 1],
                            in_=mm8[0:1, 0:4], identity=identf[0:1, 0:1])
        nc.scalar.copy(out=rhs5[0:4, :], in_=PS[0:4, PB_MM8:PB_MM8 + 1])
        nc.tensor.matmul(out=PS[0:NSEL, PB_IDC:PB_IDC + 1], lhsT=sel16,
                         rhs=rhs5[:, :], start=True, stop=True)
        idc_i = stat.tile([NSEL, 1], I32, tag="idc_i")
        nc.vector.tensor_copy(out=idc_i[:, :],
                              in_=PS[0:NSEL, PB_IDC:PB_IDC + 1])

        if LEVEL == 1:
            l1 = stat.tile([NSEL, C], F32, tag="l1")
            nc.vector.tensor_copy(out=l1[0:4, 0:8],
                                  in_=mm8[0:1, :].to_broadcast([4, 8]))
            nc.sync.dma_start(out=out[:, :], in_=l1[:, :])
            return

        # gather the 64 tokens (row 16T+4p+t = token 16*I[p] + 4T + t)
        x_sel = sb.tile([NSEL, C], F32, tag="x_sel")
        nc.gpsimd.indirect_dma_start(
            out=x_sel[:, :], out_offset=None, in_=xb[:, :],
            in_offset=bass.IndirectOffsetOnAxis(ap=idc_i[:, 0:1], axis=0))

        if LEVEL == 2:
            nc.sync.dma_start(out=out[:, :], in_=x_sel[:, :])
            return

        # ---------------- qkvg projection (fp16) ----------------
        nc.tensor.transpose(out=PS[0:128, PB_XSELT:PB_XSELT + NSEL],
                            in_=x_sel[:, :],
                            identity=identf[0:NSEL, 0:NSEL])
        x_selT = sb.tile([C, NSEL], F16, tag="x_selT")
        nc.scalar.copy(out=x_selT[:, :], in_=PS[:, PB_XSELT:PB_XSELT + NSEL])

        for g in range(8):
            nc.tensor.matmul(out=PS[0:NSEL, 512 * g:512 * (g + 1)],
                             lhsT=x_selT[:, :],
                             rhs=wqT[:, 512 * g:512 * (g + 1)],
                             start=True, stop=True)

        # staging to fp16: qk rows 0:32 by block (ACT/DVE/Pool/ACT),
        # then vg rows 32:64
        stQK = sb.tile([32, 4 * C * H], F16, tag="stQK")
        stVG = sb.tile([32, 4 * C * H], F16, tag="stVG")
        nc.scalar.copy(out=stQK[:, 0:1024], in_=PS[0:32, 0:1024])
        nc.vector.tensor_copy(out=stQK[:, 1024:2048], in_=PS[0:32, 1024:2048])
        nc.scalar.copy(out=stQK[:, 2048:3072], in_=PS[0:32, 2048:3072])
        nc.vector.tensor_copy(out=stQK[:, 3072:4096], in_=PS[0:32, 3072:4096])
        # qk rearrange: src iterates (r, b, col) matching plain dst
        # partition order 4r+b exactly
        qk = sb.tile([128, H, C], F16, tag="qk")
        FQ = 4 * C * H
        nc.sync.dma_start(
            out=qk[:, :, :],
            in_=rap(stQK[:, :], [[FQ, 32], [1024, 4], [1, 1024]]))
        nc.scalar.copy(out=stVG[:, 0:1024], in_=PS[32:64, 0:1024])
        nc.vector.tensor_copy(out=stVG[:, 1024:2048], in_=PS[32:64, 1024:2048])
        nc.scalar.copy(out=stVG[:, 2048:3072], in_=PS[32:64, 2048:3072])
        nc.vector.tensor_copy(out=stVG[:, 3072:4096], in_=PS[32:64, 3072:4096])

        # g rearrange (rows 16:32 of stVG), then v (rows 0:16) into v_sb
        g_sb = sb.tile([NSEL, H, C], F16, tag="g_sb")
        nc.sync.dma_start(
            out=g_sb[:, :, :],
            in_=rap(stVG[:, :], [[FQ, 16], [1024, 4], [1, 1024]],
                    offset=16 * FQ))
        nc.sync.dma_start(
            out=v_sb[0:NSEL, :, :],
            in_=rap(stVG[:, :], [[FQ, 16], [1024, 4], [1, 1024]]))

        if LEVEL == 3:
            l3 = sb.tile([NSEL, C], F32, tag="l3")
            nc.vector.tensor_copy(out=l3[:, :], in_=qk[0:NSEL, 0, :])
            nc.sync.dma_start(out=out[:, :], in_=l3[:, :])
            return

        # causal-mask preload for both att groups (PE idle window)
        for g in range(2):
            attb = PB_ATT0 if g == 0 else PB_ATT1
            nc.tensor.matmul(
                out=PS[0:S, attb:attb + 4 * S],
                lhsT=identh[0:S, 0:S],
                rhs=rap(t2h, [[2 * TB_COLS, 65], [0, 4], [1, 65]],
                        offset=2 * TB_CMASK),
                start=True, stop=False)

        # ---------------- rmsnorm + rope (fp16) ----------------
        # squares: ACT heads 5:8 (accum), DVE heads 0:5 (fp16 2x + reduce)
        ssq = sb.tile([128, H], F32, tag="ssq")
        sqj = junkp.tile([128, 5, C], F16, tag="sqj")
        sqa = junkp.tile([128, 3, C], F32, tag="sqa")
        for h in range(5):
            nc.vector.scalar_tensor_tensor(
                out=sqj[:, h, :], in0=qk[:, h, :], scalar=1.0,
                in1=qk[:, h, :], op0=ALU.mult, op1=ALU.mult,
                accum_out=ssq[:, h:h + 1])
        for h in range(5, 8):
            nc.scalar.activation(out=sqa[:, h - 5, :], in_=qk[:, h, :],
                                 func=AF.Square,
                                 accum_out=ssq[:, h:h + 1])
        # rope (independent of rf): r1 = qk*cos; r2 = swap(qk)*sin
        r1 = sb.tile([128, H, C], F16, tag="r1")
        r2 = sb.tile([128, H, C], F16, tag="r2")
        qkr = sb.tile([128, H, C], F16, tag="qkr")
        qkn = sb.tile([128, H, C], F16, tag="qkn")
        nc.vector.tensor_tensor(
            out=r1[:, :, :], in0=qk[:, :, :],
            in1=cosdup.rearrange("p (a c) -> p a c", a=1)
                .to_broadcast([128, H, C]), op=ALU.mult)
        nc.vector.tensor_tensor(
            out=r2[:, :, 0:64], in0=qk[:, :, 64:128],
            in1=rap(t2h, [[2 * TB_COLS, 128], [0, H], [1, 64]],
                    offset=2 * TB_SIN),
            op=ALU.mult)
        nc.vector.tensor_tensor(
            out=r2[:, :, 64:128], in0=qk[:, :, 0:64],
            in1=rap(t2h, [[2 * TB_COLS, 128], [0, H], [1, 64]],
                    offset=2 * TB_SIN + 64),
            op=ALU.mult)
        nc.vector.tensor_add(out=qkr[:, :, :], in0=r1[:, :, :],
                             in1=r2[:, :, :])
        # rf = rsqrt(ssq/C + eps) via bit-trick + 2 Newton steps, on Pool
        msv = sb.tile([128, H], F32, tag="msv")
        nwa = sb.tile([128, H], F32, tag="nwa")
        nwb = sb.tile([128, H], F32, tag="nwb")
        yv = sb.tile([128, H], F32, tag="yv")
        rf = sb.tile([128, H], F16, tag="rf")
        nc.gpsimd.tensor_scalar(out=msv[:, :], in0=ssq[:, :],
                                scalar1=1.0 / C, scalar2=EPS,
                                op0=ALU.mult, op1=ALU.add)
        msv_i = msv[:, :].bitcast(I32)
        yv_i = yv[:, :].bitcast(I32)
        nc.vector.tensor_scalar(out=yv_i, in0=msv_i, scalar1=1,
                                scalar2=None, op0=ALU.arith_shift_right)
        nc.vector.tensor_scalar(out=yv_i, in0=yv_i, scalar1=-1,
                                scalar2=MAGIC, op0=ALU.mult, op1=ALU.add)
        for it in range(2):
            eng = nc.gpsimd if it == 0 else nc.vector
            eng.tensor_tensor(out=nwa[:, :], in0=yv[:, :],
                              in1=yv[:, :], op=ALU.mult)
            eng.tensor_tensor(out=nwb[:, :], in0=nwa[:, :],
                              in1=msv[:, :], op=ALU.mult)
            eng.tensor_scalar(out=nwb[:, :], in0=nwb[:, :],
                              scalar1=-0.5, scalar2=1.5,
                              op0=ALU.mult, op1=ALU.add)
            eng.tensor_tensor(out=yv[:, :], in0=yv[:, :],
                              in1=nwb[:, :], op=ALU.mult)
        nc.vector.tensor_copy(out=rf[:, :], in_=yv[:, :])
        # qkn = qkr * rf (broadcast over c): g0 on DVE first, then g1
        nc.vector.tensor_tensor(
            out=qkn[:, 0:4, :], in0=qkr[:, 0:4, :],
            in1=rf[:, 0:4].rearrange("p (h a) -> p h a", a=1)
                .to_broadcast([128, 4, C]), op=ALU.mult)
        nc.vector.tensor_tensor(
            out=qkn[:, 4:6, :], in0=qkr[:, 4:6, :],
            in1=rf[:, 4:6].rearrange("p (h a) -> p h a", a=1)
                .to_broadcast([128, 2, C]), op=ALU.mult)
        nc.gpsimd.tensor_tensor(
            out=qkn[:, 6:8, :], in0=qkr[:, 6:8, :],
            in1=rf[:, 6:8].rearrange("p (h a) -> p h a", a=1)
                .to_broadcast([128, 2, C]), op=ALU.mult)

        if LEVEL == 4:
            l4 = sb.tile([NSEL, C], F32, tag="l4")
            nc.vector.tensor_copy(out=l4[:, :], in_=qkn[0:NSEL, 0, :])
            nc.sync.dma_start(out=out[:, :], in_=l4[:, :])
            return

        # ---------------- transposes to qnT / knT (fp16 via PSUM bitcast) --
        qnT = sb.tile([C, H, S], F16, tag="qnT")
        knT = sb.tile([C, H, S], F16, tag="knT")
        nc.scalar.copy(out=rap(qnT[:, :, :], [[H * S, C], [S, H], [1, 1]],
                               offset=NSEL),
                       in_=sinkTq.rearrange("c (h a) -> c h a", a=1))
        nc.scalar.copy(out=rap(knT[:, :, :], [[H * S, C], [S, H], [1, 1]],
                               offset=NSEL),
                       in_=sinkTk.rearrange("c (h a) -> c h a", a=1))

        if LEVEL == 41:
            l41 = sb.tile([NSEL, C], F32, tag="l41")
            nc.vector.tensor_copy(out=l41[:, 0:8],
                                  in_=qnT[0:NSEL, 0, 0:8])
            nc.vector.memset(l41[:, 8:128], 0.0)
            nc.sync.dma_start(out=out[:, :], in_=l41[:, :])
            return

        pexp = sb.tile([S, H, S], F16, tag="pexp")
        for g in range(2):
            base16 = 512 * g
            # one [128,128] transpose per head: out cols 0:64 = q^T,
            # 64:128 = k^T (tile_position (0,0); fp16 (64,0) faults)
            for j in range(4):
                h = 4 * g + j
                nc.tensor.transpose(
                    out=rap(PS16, [[8192, 128], [1, 128]],
                            offset=base16 + 128 * j),
                    in_=qkn[:, h, :],
                    identity=identh)
            for si, dstT in enumerate((qnT, knT)):
                dst = rap(dstT[:, :, :], [[H * S, C], [S, 4], [1, NSEL]],
                          offset=4 * g * S)
                src = rap(PS16, [[8192, 128], [128, 4], [1, NSEL]],
                          offset=base16 + 64 * si)
                nc.vector.tensor_copy(out=dst, in_=src)
            if LEVEL == 40 + 5 * g + 3:  # 43->g0, 48->g1
                l43 = sb.tile([NSEL, C], F32, tag="l43")
                nc.vector.memset(l43[:, :], 0.0)
                nc.vector.tensor_copy(out=l43[:, 0:65],
                                      in_=qnT[0:NSEL, 4 * g, :])
                nc.sync.dma_start(out=out[:, :], in_=l43[:, :])
                return
            # attention: QK^T accumulated onto the preloaded mask, then exp
            attb = PB_ATT0 if g == 0 else PB_ATT1
            for j in range(4):
                h = 4 * g + j
                nc.tensor.matmul(out=PS[0:S, attb + S * j:attb + S * (j + 1)],
                                 lhsT=knT[:, h, :], rhs=qnT[:, h, :],
                                 start=False, stop=(j == 3))
            nc.scalar.activation(
                out=pexp[:, 4 * g:4 * (g + 1), :],
                in_=PS[0:S, attb:attb + 4 * S].rearrange(
                    "p (h s) -> p h s", h=4),
                func=AF.Exp, bias=expb_t[:, :], scale=SCALE)
            if LEVEL == 42 + g:
                l42 = sb.tile([NSEL, C], F32, tag="l42")
                nc.vector.memset(l42[:, :], 0.0)
                nc.vector.tensor_copy(out=l42[:, 0:64],
                                      in_=pexp[0:NSEL, 4 * g, 0:64])
                nc.sync.dma_start(out=out[:, :], in_=l42[:, :])
                return
        if LEVEL == 5:
            l5 = sb.tile([NSEL, C], F32, tag="l5")
            nc.vector.tensor_copy(out=l5[:, 0:64], in_=pexp[0:NSEL, 0, 0:64])
            nc.vector.tensor_copy(out=l5[:, 64:128], in_=pexp[0:NSEL, 1, 0:64])
            nc.sync.dma_start(out=out[:, :], in_=l5[:, :])
            return

        # gT transposes + e_gT = exp(-g) (fp16)
        e_gT = sb.tile([C, H, NSEL], F16, tag="e_gT")
        for h in range(H):
            nc.tensor.transpose(
                out=rap(PS16, [[8192, 128], [1, NSEL]],
                        offset=PB_GT16 + NSEL * h),
                in_=g_sb[:, h, :],
                identity=identh[0:NSEL, 0:NSEL])
        nc.scalar.activation(
            out=e_gT[:, :, :],
            in_=rap(PS16, [[8192, 128], [NSEL, H], [1, NSEL]],
                    offset=PB_GT16).rearrange("p h s -> p h s"),
            func=AF.Exp, scale=-1.0)

        # den broadcast to all 128 partitions via ones[65,128] matmul
        nc.tensor.matmul(
            out=PS[0:128, PB_DENB:PB_DENB + 512],
            lhsT=ones65B[:, :],
            rhs=rap(pexp[:, :, :], [[H * S, S], [S, H], [1, NSEL]]),
            start=True, stop=True)

        # yT per head (unnormalized p), then yg^T = yT / ((1+e^-g)*den)
        for h in range(H):
            nc.tensor.matmul(out=PS[0:C, PB_YT + NSEL * h:
                                     PB_YT + NSEL * (h + 1)],
                             lhsT=v_sb[:, h, :], rhs=pexp[:, h, 0:NSEL],
                             start=True, stop=True)
        D = sb.tile([128, 512], F32, tag="D")
        ygT = sb.tile([C, H, NSEL], F16, tag="ygT")
        egT_flat = e_gT[:, :, :].rearrange("p h s -> p (h s)")
        ygT_flat = ygT[:, :, :].rearrange("p h s -> p (h s)")
        Dr = sb.tile([128, 512], F32, tag="Dr")
        nc.vector.scalar_tensor_tensor(
            out=D[:, 0:256], in0=egT_flat[:, 0:256],
            scalar=1.0, in1=PS[:, PB_DENB:PB_DENB + 256],
            op0=ALU.add, op1=ALU.mult)
        nc.vector.reciprocal(out=Dr[:, 0:256], in_=D[:, 0:256])
        nc.vector.tensor_tensor(
            out=ygT_flat[:, 0:256],
            in0=PS[:, PB_YT:PB_YT + 256], in1=Dr[:, 0:256], op=ALU.mult)
        nc.vector.scalar_tensor_tensor(
            out=D[:, 256:512], in0=egT_flat[:, 256:512],
            scalar=1.0, in1=PS[:, PB_DENB + 256:PB_DENB + 512],
            op0=ALU.add, op1=ALU.mult)
        nc.vector.reciprocal(out=Dr[:, 256:512], in_=D[:, 256:512])
        nc.vector.tensor_tensor(
            out=ygT_flat[:, 256:512],
            in0=PS[:, PB_YT + 256:PB_YT + 512], in1=Dr[:, 256:512],
            op=ALU.mult)

        # ---------------- output projection ----------------
        out_ps = PS[0:NSEL, PB_OUT:PB_OUT + C]
        out_sb = sb.tile([NSEL, C], F32, tag="out_sb")
        for h in range(H):
            nc.tensor.matmul(out=out_ps, lhsT=ygT[:, h, :],
                             rhs=woTb[:, h, :], start=(h == 0),
                             stop=(h == H - 1))
        nc.vector.tensor_copy(out=out_sb[:, :], in_=out_ps)
        nc.sync.dma_start(out=out[:, :], in_=out_sb[:, :])


def make_host_constants(inputs):
    """Host-side prep of tables derived from the (full) inputs."""
    cos = np.asarray(inputs["cos"]).reshape(S, 64).astype(np.float32)
    sin = np.asarray(inputs["sin"]).reshape(S, 64).astype(np.float32)
    sink = np.asarray(inputs["sink"]).reshape(H, C).astype(np.float32)
    tao = np.asarray(inputs["tao"]).astype(np.float32)
    wq = np.asarray(inputs["W_qkvg"]).astype(np.float32)
    wo = np.asarray(inputs["W_out"]).astype(np.float32)

    pos = np.arange(64) + 1
    cos_p = cos[pos]
    sin_p = sin[pos]
    cosdup = np.tile(np.concatenate([cos_p, cos_p], axis=1), (2, 1))
    sinpm = np.tile(np.concatenate([sin_p, -sin_p], axis=1), (2, 1))
    taocol = np.concatenate([np.full((64, 1), tao[0], np.float32),
                             np.full((64, 1), tao[1], np.float32)])
    cosdup = (cosdup * taocol).astype(np.float16)
    sinpm = (sinpm * taocol).astype(np.float16)

    # additive causal mask TRANSPOSED: mask[t, s] = 0 if pos_t <= pos_s
    posf = np.where(np.arange(S) < NSEL, np.arange(S) + 1, 0)
    cmaskm = np.where(posf[:, None] <= posf[None, :], 0.0,
                      NEG_BIG).astype(np.float16)  # [t, s]

    sn = sink / np.sqrt((sink * sink).mean(axis=-1, keepdims=True) + EPS)
    sinkTq = np.ascontiguousarray((sn * tao[0]).T).astype(np.float16)
    sinkTk = np.ascontiguousarray((sn * tao[1]).T).astype(np.float16)
    sinkv = sink.reshape(1, H * C).astype(np.float16)

    # sel16'[p, r] = -16 if p == p(r); sel16'[4, r] = 16*NP + 4*T(r) + t(r)
    sel16m = np.zeros((5, NSEL), np.float32)
    for Tn in range(4):
        for p in range(4):
            for t in range(4):
                r = 16 * Tn + 4 * p + t
                sel16m[p, r] = -16.0
                sel16m[4, r] = float(16 * NP + 4 * Tn + t)

    negio = (float(NP) - np.arange(NP, dtype=np.float32)).astype(np.float16)

    wqT = np.ascontiguousarray(wq.T).astype(np.float16)
    woT = np.ascontiguousarray(
        wo.reshape(C, H, C).transpose(2, 1, 0)).astype(np.float16)

    identf32 = np.eye(128, dtype=np.float32)
    identf16 = np.eye(128, dtype=np.float16)

    def pack16(a, rows=128):
        c = a.shape[1]
        padded = np.zeros((128, c), np.float16)
        padded[:rows] = a
        return padded.view(np.float32)

    tabs2 = np.zeros((128, TB_COLS), np.float32)
    tabs2[0:5, TB_SEL16:TB_SEL16 + 64] = sel16m
    tabs2[:, TB_IDF16:TB_IDF16 + 64] = pack16(identf16)
    tabs2[:, TB_COS:TB_COS + 64] = pack16(cosdup)
    tabs2[:, TB_SIN:TB_SIN + 64] = pack16(sinpm)
    tabs2[:, TB_SINKQ:TB_SINKQ + 4] = pack16(sinkTq)
    tabs2[:, TB_SINKK:TB_SINKK + 4] = pack16(sinkTk)
    cm = np.zeros((65, 66), np.float16)
    cm[:, 0:65] = cmaskm
    tabs2[:, TB_CMASK:TB_CMASK + 33] = pack16(cm, rows=65)
    tabs2[:, TB_NEGIO:TB_NEGIO + 256] = pack16(negio.reshape(1, NP), rows=1)
    return dict(tabs2=tabs2, identd=identf32, sinkv=sinkv,
                wqT_d=wqT, woT_d=woT)


_CACHE = {}


def get_nc():
    if "nc" not in _CACHE:
        nc = bacc.Bacc("TRN2", target_bir_lowering=False, debug=False,
                       num_devices=B)
        build_kernel(nc)
        nc.compile()
        _CACHE["nc"] = nc
    return _CACHE["nc"]


def make_in_maps(inputs):
    x = np.ascontiguousarray(inputs["x"], dtype=np.float32)
    pwv = np.concatenate(
        [np.asarray(inputs["patch_w"], np.float32).ravel(),
         np.ones(128, np.float32)]).reshape(1, PATCH + 128)
    consts = make_host_constants(inputs)
    in_maps = []
    for b in range(B):
        m = {"xb": np.ascontiguousarray(x[b]), "pw": pwv}
        m.update(consts)
        in_maps.append(m)
    return in_maps


def kernel(**inputs):
    nc = get_nc()
    in_maps = make_in_maps(inputs)
    res = run_bass_kernel_spmd(nc, in_maps, core_ids=list(range(B)))
    return np.stack([r["out"] for r in res.results], axis=0)


if __name__ == "__main__":
    nc = get_nc()
    print("build ok:", len(nc.m.functions[0].allocations), "allocations")


# revision 36
# speedup vs baseline: 1.1016x; 1.0381x over previous
"""Trainium2 Bass kernel for nn_AttentionOnDetail (sparse patch attention).

Data-parallel over batch B=8 across 8 NeuronCores; one batch per core.
v3 redesign (latency-focused; the kernel is dependency-bound):
  - x streamed first (x tile DMAs are the first SP descriptors; tile 3 in
    four 512-col chunks).  pw row + f32 ident ride gpsimd SWDGE so their
    transfers slip into the stream right after tile 0.
  - pw broadcast stays in PSUM (dot stt reads PSUM directly, no copies);
    PE warmup matmuls run on the eps tile at t~0.9 so everything after
    runs at peak p-state.
  - stats split across engines: ACT squares (t0-t2, 3a, 3b, 3d), DVE dots
    (t0-t2, 3c, 3d) + logit chain, Pool dots (3a, 3b) + square (3c).
  - patch logits use the monotone transform dot*|dot|/ms (no Sqrt); the
    selection runs once globally: top8 -> threshold mask * negio -> top8
    gives the ranks directly (idc matmul folds NP-v and the *16).
  - single ACT function set (Exp/Square/Copy, set 0) loaded once at t=0;
    rmsnorm rsqrt = Newton iteration on Pool (bit-trick seed), sigmoid
    via exp(-g), softmax exp with folded -6 bias so p fits fp16.
  - everything from the projection on runs in fp16 (W cast on host).
  - attention computed transposed (att_T = k^T q) with the causal mask
    preloaded into PSUM via an identity matmul; denominator broadcast to
    all partitions by a single ones[65,128] matmul; gate folded as
    yg = y / ((1+e^-g)*den) with DVE/Pool divide; output projection
    consumes yg^T directly.
"""

import sys
import numpy as np

for _p in ("/opt/trn_rl_repo",):
    if _p not in sys.path:
        sys.path.insert(0, _p)

import concourse.bass as bass
import concourse.bacc as bacc
import concourse.tile as tile
from concourse import mybir
from concourse.bass_utils import run_bass_kernel_spmd

F32 = mybir.dt.float32
F32R = mybir.dt.float32r
F16 = mybir.dt.float16
I32 = mybir.dt.int32
U32 = mybir.dt.uint32
AF = mybir.ActivationFunctionType
ALU = mybir.AluOpType
AX = mybir.AxisListType

B, T, C, H, T0 = 8, 8192, 128, 8, 16
NP = T // T0          # 512 patches
PATCH = T0 * C        # 2048 elements per patch
S = 65                # sink + 64 selected tokens
NSEL = 64
EPS = 1.1920929e-07
SCALE = 1.0 / float(np.sqrt(np.float32(C)))
EXPB = -6.0           # softmax exp bias; den-normalization cancels it
NEG_BIG = -60000.0    # additive causal mask (fp16-representable)
MAGIC = 0x5F3759DF    # fast-rsqrt seed

# tabs2 f32-column layout (fp16 payloads packed as pairs into f32 cols)
TB_SEL16 = 0          # sel16' f32 [5, 64]
TB_IDF16 = 64         # ident f16 [128, 128] -> 64 f32 cols
TB_COS = 128          # cosdup f16 [128, 128] -> 64
TB_SIN = 192          # sinpm f16 [128, 128] -> 64
TB_SINKQ = 256        # sinkTq f16 [128, 8] -> 4
TB_SINKK = 260        # sinkTk f16 [128, 8] -> 4
TB_CMASK = 264        # cmaskT f16 [65, 66] -> 33 (col 65 pad)
TB_NEGIO = 297        # negio f16 [1, 512] -> 256
TB_COLS = 553

# PSUM f32-column region plan (8 banks x 512 cols)
PB_PWB = 0            # pwB broadcast [128, 2048] (cols 0:2048), early only
PB_QNT16 = 0          # qkn transposes (f16 cols 0:1024 = f32 0:512)
PB_GT16 = 2048        # gT transposes (f16 cols 2048:2560 = f32 1024:1280)
PB_YT = 1536          # yT [128, (h,s)=512] cols 1536:2048 (bank 3)
PB_OUT = 3584         # out [64, 128] in bank 7 (logits row dead)
PB_ATT0 = 2048        # att_T group 0 [65, 260]
PB_ATT1 = 2560        # att_T group 1 [65, 260]
PB_XSELT = 3072       # x_selT staging [128, 64]
PB_DENB = 3072        # den broadcast [128, 512] (after x_selT dead)
PB_WARM = 3500        # warmup scratch col
PB_MM8 = 3582         # mm8 transpose column [4, 1]
PB_IDC = 3583         # sel16 matmul out [64, 1]
LROW = 3584           # logits row [1, 512] (rows 0:1)


def rap(t, apl, offset=0):
    """Raw AP over a tile/AP's storage, flat element strides."""
    base = t if isinstance(t, bass.AP) else t[:]
    return bass.AP(tensor=base.tensor, offset=base.offset + offset,
                   ap=[list(x) for x in apl])


def build_kernel(nc):
    xb = nc.dram_tensor("xb", [T, C], F32, kind="ExternalInput")
    pw = nc.dram_tensor("pw", [1, PATCH + 128], F32R, kind="ExternalInput")
    identd = nc.dram_tensor("identd", [128, 128], F32, kind="ExternalInput")
    tabs2 = nc.dram_tensor("tabs2", [128, TB_COLS], F32, kind="ExternalInput")
    wqT_d = nc.dram_tensor("wqT_d", [C, 4 * C * H], F16, kind="ExternalInput")
    woT_d = nc.dram_tensor("woT_d", [C, H, C], F16, kind="ExternalInput")
    sinkv = nc.dram_tensor("sinkv", [1, H * C], F16, kind="ExternalInput")
    out = nc.dram_tensor("out", [NSEL, C], F32, kind="ExternalOutput")

    with tile.TileContext(nc) as tc:
        _emit(tc, nc, xb, pw, identd, tabs2, wqT_d, woT_d, sinkv, out)
    return nc


def _emit(tc, nc, xb, pw, identd, tabs2_d, wqT_d, woT_d, sinkv, out):
    import os
    LEVEL = int(os.environ.get("KLEVEL", "9"))
    from contextlib import ExitStack
    ctx = ExitStack()
    with ctx:
        const1 = ctx.enter_context(tc.tile_pool(name="const1", bufs=1))
        xpool = ctx.enter_context(tc.tile_pool(name="xpool", bufs=1))
        junkp = ctx.enter_context(tc.tile_pool(name="junkp", bufs=1))
        stat = ctx.enter_context(tc.tile_pool(name="stat", bufs=4))
        sb = ctx.enter_context(tc.tile_pool(name="sb", bufs=1))
        psall = ctx.enter_context(tc.tile_pool(name="psall", bufs=1,
                                               space="PSUM"))
        PS = psall.tile([128, 4096], F32)
        PS16 = PS[:, :].bitcast(F16)  # [128, 8192] f16 view

        # ---------------- x stream first; pw/ident via gpsimd --------------
        def xdma(i):
            xp = xpool.tile([128, PATCH], F32, tag=f"xp{i}")
            nc.sync.dma_start(
                out=xp[:, :],
                in_=rap(xb[:, :], [[PATCH, 128], [1, PATCH]],
                        offset=i * 128 * PATCH))
            return xp

        xps = [xdma(0)]
        pwo_sb = const1.tile([1, PATCH + 128], F32R)
        nc.gpsimd.dma_start(out=pwo_sb[:, :], in_=pw[:, :])
        identf = const1.tile([128, 128], F32)
        nc.gpsimd.dma_start(out=identf[:, :], in_=identd[:, :])
        xps.append(xdma(1))
        xps.append(xdma(2))
        xp3 = xpool.tile([128, PATCH], F32, tag="xp3")
        for ch in range(4):
            nc.sync.dma_start(
                out=xp3[:, 512 * ch:512 * (ch + 1)],
                in_=rap(xb[:, :], [[PATCH, 128], [1, 512]],
                        offset=3 * 128 * PATCH + 512 * ch))
        xps.append(xp3)

        tabs2 = const1.tile([128, TB_COLS], F32)
        nc.sync.dma_start(out=tabs2[:, :], in_=tabs2_d[:, :])
        t2h = tabs2[:, :].bitcast(F16)  # [128, 2*TB_COLS] f16 view

        def h16(col_f32, ncols_f16, nrows=128):
            return rap(t2h, [[2 * TB_COLS, nrows], [1, ncols_f16]],
                       offset=2 * col_f32)

        identh_v = h16(TB_IDF16, 128)
        identh = None  # materialized below after tabs2 lands
        cosdup = h16(TB_COS, 128)
        sinkTq = h16(TB_SINKQ, 8)
        sinkTk = h16(TB_SINKK, 8)
        negio = h16(TB_NEGIO, 512, nrows=1)
        sel16 = tabs2[0:5, TB_SEL16:TB_SEL16 + 64]

        wqT = const1.tile([C, 4 * C * H], F16)
        for wch in range(4):
            nc.sync.dma_start(out=wqT[:, 1024 * wch:1024 * (wch + 1)],
                              in_=wqT_d[:, 1024 * wch:1024 * (wch + 1)])
        woTb = const1.tile([C, H, C], F16)
        nc.sync.dma_start(out=woTb[:, :, :], in_=woT_d[:, :, :])
        v_sb = sb.tile([S, H, C], F16, tag="v_sb")
        nc.sync.dma_start(
            out=v_sb[NSEL:S, :, :],
            in_=sinkv[:, :].rearrange("p (h c) -> p h c", h=H))

        identh_t = const1.tile([128, 128], F16)
        nc.scalar.copy(out=identh_t[:, :], in_=identh_v)
        identh = identh_t[:, :]
        eps_t = const1.tile([128, 1], F32)
        nc.vector.memset(eps_t[:, :], EPS)
        expb_t = const1.tile([S, 1], F32)
        nc.vector.memset(expb_t[:, :], EXPB)
        ones65B = const1.tile([S, C], F16)
        nc.vector.memset(ones65B[:, :], 1.0)
        rhs5 = const1.tile([5, 1], F32)
        nc.vector.memset(rhs5[:, :], 1.0)
        # preload ACT set 0 (Exp/Square/Copy) once, while DMAs stream
        dummy = stat.tile([1, 1], F32)
        nc.vector.memset(dummy[:, :], 1.0)
        nc.scalar.activation(out=dummy[:, :], in_=dummy[:, :], func=AF.Exp)

        # PE warmups on the eps tile (lift p-state early)
        nc.tensor.matmul(out=PS[0:1, PB_WARM:PB_WARM + 1], lhsT=eps_t[:, :],
                         rhs=eps_t[:, :], start=True, stop=True)
        nc.tensor.matmul(out=PS[0:1, PB_WARM:PB_WARM + 1], lhsT=eps_t[:, :],
                         rhs=eps_t[:, :], start=True, stop=True)
        # pwB broadcast via K=1 matmuls; dots read it from PSUM directly
        ones_t = pwo_sb[0:1, PATCH:PATCH + 128]
        for q4 in range(4):
            nc.tensor.matmul(out=PS[:, PB_PWB + 512 * q4:
                                    PB_PWB + 512 * (q4 + 1)],
                             lhsT=ones_t,
                             rhs=pwo_sb[0:1, 512 * q4:512 * (q4 + 1)],
                             start=True, stop=True)

        # ---------------- phase 1: per-patch stats ----------------
        junk = junkp.tile([128, PATCH], F32, tag="junk")
        junk2 = junkp.tile([128, PATCH], F32, tag="junk2")
        junk3 = junkp.tile([128, PATCH], F32, tag="junk3")
        junk23 = junkp.tile([128, PATCH], F32, tag="junk23")
        ss_c = stat.tile([128, 4], F32, tag="ss_c")
        dot_c = stat.tile([128, 4], F32, tag="dot_c")
        ss3 = stat.tile([128, 4], F32, tag="ss3")
        dot3 = stat.tile([128, 4], F32, tag="dot3")
        msx = stat.tile([128, 4], F32, tag="msx")
        dd = stat.tile([128, 4], F32, tag="dd")
        logit_c = stat.tile([128, 4], F32, tag="logit_c")

        nd = stat.tile([128, 4], F32, tag="nd")
        rms = stat.tile([128, 4], F32, tag="rms")

        def logit_tile(i, eng):
            # ms = ss/PATCH + EPS; logit' = dot*|dot| * recip(ms) (order-eq)
            eng.tensor_scalar(
                out=msx[:, i:i + 1], in0=ss_c[:, i:i + 1],
                scalar1=1.0 / PATCH, scalar2=EPS, op0=ALU.mult, op1=ALU.add)
            nc.vector.reciprocal(out=rms[:, i:i + 1], in_=msx[:, i:i + 1])
            nc.vector.tensor_scalar(
                out=nd[:, i:i + 1].bitcast(I32),
                in0=dot_c[:, i:i + 1].bitcast(I32),
                scalar1=0x7FFFFFFF, scalar2=None, op0=ALU.bitwise_and)
            eng.tensor_tensor(
                out=dd[:, i:i + 1], in0=nd[:, i:i + 1],
                in1=dot_c[:, i:i + 1], op=ALU.mult)
            eng.tensor_tensor(
                out=logit_c[:, i:i + 1], in0=dd[:, i:i + 1],
                in1=rms[:, i:i + 1], op=ALU.mult)
            nc.tensor.transpose(
                out=PS[0:1, LROW + 128 * i:LROW + 128 * (i + 1)],
                in_=logit_c[:, i:i + 1], identity=identf[:, :])

        for i in range(3):
            xp = xps[i]
            nc.scalar.activation(out=junk[:, :], in_=xp[:, :],
                                 func=AF.Square,
                                 accum_out=ss_c[:, i:i + 1])
            nc.vector.scalar_tensor_tensor(
                out=junk2[:, :], in0=xp[:, :], scalar=1.0,
                in1=PS[:, PB_PWB:PB_PWB + PATCH],
                op0=ALU.mult, op1=ALU.mult,
                accum_out=dot_c[:, i:i + 1])
            logit_tile(i, nc.gpsimd)

        # tile 3 chunks: ACT squares a,b,d + Pool square c;
        # Pool dots a,b + DVE dots c,d
        def sq3(eng, ch):
            cs = slice(512 * ch, 512 * (ch + 1))
            if eng is nc.scalar:
                nc.scalar.activation(out=junk3[:, cs], in_=xp3[:, cs],
                                     func=AF.Square,
                                     accum_out=ss3[:, ch:ch + 1])
            else:
                eng.scalar_tensor_tensor(
                    out=junk3[:, cs], in0=xp3[:, cs], scalar=1.0,
                    in1=xp3[:, cs], op0=ALU.mult, op1=ALU.mult,
                    accum_out=ss3[:, ch:ch + 1])

        def dot3f(eng, ch):
            cs = slice(512 * ch, 512 * (ch + 1))
            nc.vector.scalar_tensor_tensor(
                out=junk23[:, cs], in0=xp3[:, cs], scalar=1.0,
                in1=PS[:, PB_PWB + 512 * ch:PB_PWB + 512 * (ch + 1)],
                op0=ALU.mult, op1=ALU.mult,
                accum_out=dot3[:, ch:ch + 1])

        dot3f(nc.vector, 0)
        sq3(nc.scalar, 0)
        dot3f(nc.vector, 1)
        sq3(nc.scalar, 1)
        dot3f(nc.vector, 2)
        sq3(nc.scalar, 2)
        dot3f(nc.vector, 3)
        sq3(nc.scalar, 3)
        nc.vector.tensor_reduce(out=ss_c[:, 3:4],
                                in_=ss3[:, :].rearrange("p (a f) -> p a f",
                                                        a=1),
                                axis=AX.X, op=ALU.add)
        nc.vector.tensor_reduce(out=dot_c[:, 3:4],
                                in_=dot3[:, :].rearrange("p (a f) -> p a f",
                                                         a=1),
                                axis=AX.X, op=ALU.add)
        logit_tile(3, nc.vector)

        # ---------------- top-4 selection (global, on the PSUM row) --------
        lrow = PS[0:1, LROW:LROW + NP]
        gmax8 = stat.tile([1, 8], F32, tag="gmax8")
        nc.vector.max(out=gmax8[:, :], in_=lrow)
        masked = stat.tile([1, NP], F32, tag="masked")
        nc.vector.scalar_tensor_tensor(
            out=masked[:, :], in0=lrow, scalar=gmax8[:, 3:4],
            in1=negio, op0=ALU.is_ge, op1=ALU.mult)
        mm8 = stat.tile([1, 8], F32, tag="mm8")
        nc.vector.max(out=mm8[:, :], in_=masked[:, :])

        # patch ranks (NP - v) fold into sel16'; rhs = [v0..v3, 1]
        nc.tensor.transpose(out=PS[0:4, PB_MM8:PB_MM8 + 1],
                            in_=mm8[0:1, 0:4], identity=identf[0:1, 0:1])
        nc.scalar.copy(out=rhs5[0:4, :], in_=PS[0:4, PB_MM8:PB_MM8 + 1])
        nc.tensor.matmul(out=PS[0:NSEL, PB_IDC:PB_IDC + 1], lhsT=sel16,
                         rhs=rhs5[:, :], start=True, stop=True)
        idc_i = stat.tile([NSEL, 1], I32, tag="idc_i")
        nc.vector.tensor_copy(out=idc_i[:, :],
                              in_=PS[0:NSEL, PB_IDC:PB_IDC + 1])

        if LEVEL == 1:
            l1 = stat.tile([NSEL, C], F32, tag="l1")
            nc.vector.tensor_copy(out=l1[0:4, 0:8],
                                  in_=mm8[0:1, :].to_broadcast([4, 8]))
            nc.sync.dma_start(out=out[:, :], in_=l1[:, :])
            return

        # gather the 64 tokens (row 16T+4p+t = token 16*I[p] + 4T + t)
        x_sel = sb.tile([NSEL, C], F32, tag="x_sel")
        nc.gpsimd.indirect_dma_start(
            out=x_sel[:, :], out_offset=None, in_=xb[:, :],
            in_offset=bass.IndirectOffsetOnAxis(ap=idc_i[:, 0:1], axis=0))

        if LEVEL == 2:
            nc.sync.dma_start(out=out[:, :], in_=x_sel[:, :])
            return

        # ---------------- qkvg projection (fp16) ----------------
        nc.tensor.transpose(out=PS[0:128, PB_XSELT:PB_XSELT + NSEL],
                            in_=x_sel[:, :],
                            identity=identf[0:NSEL, 0:NSEL])
        x_selT = sb.tile([C, NSEL], F16, tag="x_selT")
        nc.scalar.copy(out=x_selT[:, :], in_=PS[:, PB_XSELT:PB_XSELT + NSEL])

        for g in range(8):
            nc.tensor.matmul(out=PS[0:NSEL, 512 * g:512 * (g + 1)],
                             lhsT=x_selT[:, :],
                             rhs=wqT[:, 512 * g:512 * (g + 1)],
                             start=True, stop=True)

        # staging to fp16: qk rows 0:32 by block (ACT/DVE/Pool/ACT),
        # then vg rows 32:64
        stQK = sb.tile([32, 4 * C * H], F16, tag="stQK")
        stVG = sb.tile([32, 4 * C * H], F16, tag="stVG")
        nc.scalar.copy(out=stQK[:, 0:1024], in_=PS[0:32, 0:1024])
        nc.vector.tensor_copy(out=stQK[:, 1024:2048], in_=PS[0:32, 1024:2048])
        nc.scalar.copy(out=stQK[:, 2048:3072], in_=PS[0:32, 2048:3072])
        nc.vector.tensor_copy(out=stQK[:, 3072:4096], in_=PS[0:32, 3072:4096])
        # qk rearrange: src iterates (r, b, col) matching plain dst
        # partition order 4r+b exactly
        qk = sb.tile([128, H, C], F16, tag="qk")
        FQ = 4 * C * H
        nc.sync.dma_start(
            out=qk[:, :, :],
            in_=rap(stQK[:, :], [[FQ, 32], [1024, 4], [1, 1024]]))
        nc.scalar.copy(out=stVG[:, 0:1024], in_=PS[32:64, 0:1024])
        nc.vector.tensor_copy(out=stVG[:, 1024:2048], in_=PS[32:64, 1024:2048])
        nc.scalar.copy(out=stVG[:, 2048:3072], in_=PS[32:64, 2048:3072])
        nc.vector.tensor_copy(out=stVG[:, 3072:4096], in_=PS[32:64, 3072:4096])

        # g rearrange (rows 16:32 of stVG), then v (rows 0:16) into v_sb
        g_sb = sb.tile([NSEL, H, C], F16, tag="g_sb")
        nc.sync.dma_start(
            out=g_sb[:, :, :],
            in_=rap(stVG[:, :], [[FQ, 16], [1024, 4], [1, 1024]],
                    offset=16 * FQ))
        nc.sync.dma_start(
            out=v_sb[0:NSEL, :, :],
            in_=rap(stVG[:, :], [[FQ, 16], [1024, 4], [1, 1024]]))

        if LEVEL == 3:
            l3 = sb.tile([NSEL, C], F32, tag="l3")
            nc.vector.tensor_copy(out=l3[:, :], in_=qk[0:NSEL, 0, :])
            nc.sync.dma_start(out=out[:, :], in_=l3[:, :])
            return

        # causal-mask preload for both att groups (PE idle window)
        for g in range(2):
            attb = PB_ATT0 if g == 0 else PB_ATT1
            nc.tensor.matmul(
                out=PS[0:S, attb:attb + 4 * S],
                lhsT=identh[0:S, 0:S],
                rhs=rap(t2h, [[2 * TB_COLS, 65], [0, 4], [1, 65]],
                        offset=2 * TB_CMASK),
                start=True, stop=False)

        # ---------------- rmsnorm + rope (fp16) ----------------
        # squares: ACT heads 5:8 (accum), DVE heads 0:5 (fp16 2x + reduce)
        ssq = sb.tile([128, H], F32, tag="ssq")
        sqj = junkp.tile([128, 6, C], F16, tag="sqj")
        sqa = junkp.tile([128, 2, C], F32, tag="sqa")
        for h in range(6):
            nc.vector.scalar_tensor_tensor(
                out=sqj[:, h, :], in0=qk[:, h, :], scalar=1.0,
                in1=qk[:, h, :], op0=ALU.mult, op1=ALU.mult,
                accum_out=ssq[:, h:h + 1])
        for h in range(6, 8):
            nc.scalar.activation(out=sqa[:, h - 6, :], in_=qk[:, h, :],
                                 func=AF.Square,
                                 accum_out=ssq[:, h:h + 1])
        # rf = rsqrt(ssq/C + eps): bit-trick seed (DVE) + 2 Newton
        # steps on Pool, overlapped with rope on DVE
        msv = sb.tile([128, H], F32, tag="msv")
        nwa = sb.tile([128, H], F32, tag="nwa")
        nwb = sb.tile([128, H], F32, tag="nwb")
        yv = sb.tile([128, H], F32, tag="yv")
        rf = sb.tile([128, H], F16, tag="rf")
        nc.gpsimd.tensor_scalar(out=msv[:, :], in0=ssq[:, :],
                                scalar1=1.0 / C, scalar2=EPS,
                                op0=ALU.mult, op1=ALU.add)
        msv_i = msv[:, :].bitcast(I32)
        yv_i = yv[:, :].bitcast(I32)
        nc.vector.tensor_scalar(out=yv_i, in0=msv_i, scalar1=1,
                                scalar2=None, op0=ALU.arith_shift_right)
        nc.vector.tensor_scalar(out=yv_i, in0=yv_i, scalar1=-1,
                                scalar2=MAGIC, op0=ALU.mult, op1=ALU.add)
        for it in range(2):
            nc.gpsimd.tensor_tensor(out=nwa[:, :], in0=yv[:, :],
                                    in1=yv[:, :], op=ALU.mult)
            nc.gpsimd.tensor_tensor(out=nwb[:, :], in0=nwa[:, :],
                                    in1=msv[:, :], op=ALU.mult)
            nc.gpsimd.tensor_scalar(out=nwb[:, :], in0=nwb[:, :],
                                    scalar1=-0.5, scalar2=1.5,
                                    op0=ALU.mult, op1=ALU.add)
            nc.gpsimd.tensor_tensor(out=yv[:, :], in0=yv[:, :],
                                    in1=nwb[:, :], op=ALU.mult)
        nc.gpsimd.tensor_copy(out=rf[:, :], in_=yv[:, :])
        # rope (independent of rf): r1 = qk*cos; r2 = swap(qk)*sin
        r1 = sb.tile([128, H, C], F16, tag="r1")
        r2 = sb.tile([128, H, C], F16, tag="r2")
        qkr = sb.tile([128, H, C], F16, tag="qkr")
        qkn = sb.tile([128, H, C], F16, tag="qkn")
        nc.vector.tensor_tensor(
            out=r1[:, :, :], in0=qk[:, :, :],
            in1=cosdup.rearrange("p (a c) -> p a c", a=1)
                .to_broadcast([128, H, C]), op=ALU.mult)
        nc.vector.tensor_tensor(
            out=r2[:, :, 0:64], in0=qk[:, :, 64:128],
            in1=rap(t2h, [[2 * TB_COLS, 128], [0, H], [1, 64]],
                    offset=2 * TB_SIN),
            op=ALU.mult)
        nc.vector.tensor_tensor(
            out=r2[:, :, 64:128], in0=qk[:, :, 0:64],
            in1=rap(t2h, [[2 * TB_COLS, 128], [0, H], [1, 64]],
                    offset=2 * TB_SIN + 64),
            op=ALU.mult)
        nc.vector.tensor_add(out=qkr[:, :, :], in0=r1[:, :, :],
                             in1=r2[:, :, :])
        # qkn = qkr * rf (broadcast over c): g0 on DVE first, then g1
        nc.vector.tensor_tensor(
            out=qkn[:, 0:4, :], in0=qkr[:, 0:4, :],
            in1=rf[:, 0:4].rearrange("p (h a) -> p h a", a=1)
                .to_broadcast([128, 4, C]), op=ALU.mult)
        nc.vector.tensor_tensor(
            out=qkn[:, 4:6, :], in0=qkr[:, 4:6, :],
            in1=rf[:, 4:6].rearrange("p (h a) -> p h a", a=1)
                .to_broadcast([128, 2, C]), op=ALU.mult)
        nc.gpsimd.tensor_tensor(
            out=qkn[:, 6:8, :], in0=qkr[:, 6:8, :],
            in1=rf[:, 6:8].rearrange("p (h a) -> p h a", a=1)
                .to_broadcast([128, 2, C]), op=ALU.mult)

        if LEVEL == 4:
            l4 = sb.tile([NSEL, C], F32, tag="l4")
            nc.vector.tensor_copy(out=l4[:, :], in_=qkn[0:NSEL, 0, :])
            nc.sync.dma_start(out=out[:, :], in_=l4[:, :])
            return

        # ---------------- transposes to qnT / knT (fp16 via PSUM bitcast) --
        qnT = sb.tile([C, H, S], F16, tag="qnT")
        knT = sb.tile([C, H, S], F16, tag="knT")
        nc.scalar.copy(out=rap(qnT[:, :, :], [[H * S, C], [S, H], [1, 1]],
                               offset=NSEL),
                       in_=sinkTq.rearrange("c (h a) -> c h a", a=1))
        nc.scalar.copy(out=rap(knT[:, :, :], [[H * S, C], [S, H], [1, 1]],
                               offset=NSEL),
                       in_=sinkTk.rearrange("c (h a) -> c h a", a=1))

        if LEVEL == 41:
            l41 = sb.tile([NSEL, C], F32, tag="l41")
            nc.vector.tensor_copy(out=l41[:, 0:8],
                                  in_=qnT[0:NSEL, 0, 0:8])
            nc.vector.memset(l41[:, 8:128], 0.0)
            nc.sync.dma_start(out=out[:, :], in_=l41[:, :])
            return

        pexp = sb.tile([S, H, S], F16, tag="pexp")
        for g in range(2):
            base16 = 512 * g
            # one [128,128] transpose per head: out cols 0:64 = q^T,
            # 64:128 = k^T (tile_position (0,0); fp16 (64,0) faults)
            for j in range(4):
                h = 4 * g + j
                nc.tensor.transpose(
                    out=rap(PS16, [[8192, 128], [1, 128]],
                            offset=base16 + 128 * j),
                    in_=qkn[:, h, :],
                    identity=identh)
            for si, dstT in enumerate((qnT, knT)):
                dst = rap(dstT[:, :, :], [[H * S, C], [S, 4], [1, NSEL]],
                          offset=4 * g * S)
                src = rap(PS16, [[8192, 128], [128, 4], [1, NSEL]],
                          offset=base16 + 64 * si)
                nc.vector.tensor_copy(out=dst, in_=src)
            if LEVEL == 40 + 5 * g + 3:  # 43->g0, 48->g1
                l43 = sb.tile([NSEL, C], F32, tag="l43")
                nc.vector.memset(l43[:, :], 0.0)
                nc.vector.tensor_copy(out=l43[:, 0:65],
                                      in_=qnT[0:NSEL, 4 * g, :])
                nc.sync.dma_start(out=out[:, :], in_=l43[:, :])
                return
            # attention: QK^T accumulated onto the preloaded mask, then exp
            attb = PB_ATT0 if g == 0 else PB_ATT1
            for j in range(4):
                h = 4 * g + j
                nc.tensor.matmul(out=PS[0:S, attb + S * j:attb + S * (j + 1)],
                                 lhsT=knT[:, h, :], rhs=qnT[:, h, :],
                                 start=False, stop=(j == 3))
            nc.scalar.activation(
                out=pexp[:, 4 * g:4 * (g + 1), :],
                in_=PS[0:S, attb:attb + 4 * S].rearrange(
                    "p (h s) -> p h s", h=4),
                func=AF.Exp, bias=expb_t[:, :], scale=SCALE)
            denb = 512 if g == 0 else PB_DENB
            nc.tensor.matmul(
                out=PS[0:128, denb:denb + 256],
                lhsT=ones65B[:, :],
                rhs=rap(pexp[:, :, :], [[H * S, S], [S, 4], [1, NSEL]],
                        offset=4 * g * S),
                start=True, stop=True)
            for j in range(4):
                nc.tensor.matmul(out=PS[0:C, PB_YT + NSEL * (4 * g + j):
                                         PB_YT + NSEL * (4 * g + j + 1)],
                                 lhsT=v_sb[:, 4 * g + j, :],
                                 rhs=pexp[:, 4 * g + j, 0:NSEL],
                                 start=True, stop=True)
            if LEVEL == 42 + g:
                l42 = sb.tile([NSEL, C], F32, tag="l42")
                nc.vector.memset(l42[:, :], 0.0)
                nc.vector.tensor_copy(out=l42[:, 0:64],
                                      in_=pexp[0:NSEL, 4 * g, 0:64])
                nc.sync.dma_start(out=out[:, :], in_=l42[:, :])
                return
        if LEVEL == 5:
            l5 = sb.tile([NSEL, C], F32, tag="l5")
            nc.vector.tensor_copy(out=l5[:, 0:64], in_=pexp[0:NSEL, 0, 0:64])
            nc.vector.tensor_copy(out=l5[:, 64:128], in_=pexp[0:NSEL, 1, 0:64])
            nc.sync.dma_start(out=out[:, :], in_=l5[:, :])
            return

        # gT transposes + e_gT = exp(-g) (fp16)
        e_gT = sb.tile([C, H, NSEL], F16, tag="e_gT")
        for h in range(H):
            nc.tensor.transpose(
                out=rap(PS16, [[8192, 128], [1, NSEL]],
                        offset=PB_GT16 + NSEL * h),
                in_=g_sb[:, h, :],
                identity=identh[0:NSEL, 0:NSEL])
        nc.scalar.activation(
            out=e_gT[:, :, :],
            in_=rap(PS16, [[8192, 128], [NSEL, H], [1, NSEL]],
                    offset=PB_GT16).rearrange("p h s -> p h s"),
            func=AF.Exp, scale=-1.0)

        # yg^T = yT / ((1+e^-g)*den), per group; then output projection
        D = sb.tile([128, 512], F32, tag="D")
        ygT = sb.tile([C, H, NSEL], F16, tag="ygT")
        egT_flat = e_gT[:, :, :].rearrange("p h s -> p (h s)")
        ygT_flat = ygT[:, :, :].rearrange("p h s -> p (h s)")
        Dr = sb.tile([128, 512], F32, tag="Dr")
        out_ps = PS[0:NSEL, PB_OUT:PB_OUT + C]
        out_sb = sb.tile([NSEL, C], F32, tag="out_sb")
        for g in range(2):
            cs = slice(256 * g, 256 * (g + 1))
            denb = 512 if g == 0 else PB_DENB
            nc.vector.scalar_tensor_tensor(
                out=D[:, cs], in0=egT_flat[:, cs],
                scalar=1.0, in1=PS[:, denb:denb + 256],
                op0=ALU.add, op1=ALU.mult)
            nc.vector.reciprocal(out=Dr[:, cs], in_=D[:, cs])
            nc.vector.tensor_tensor(
                out=ygT_flat[:, cs],
                in0=PS[:, PB_YT + 256 * g:PB_YT + 256 * (g + 1)],
                in1=Dr[:, cs], op=ALU.mult)
            for j in range(4):
                h = 4 * g + j
                nc.tensor.matmul(out=out_ps, lhsT=ygT[:, h, :],
                                 rhs=woTb[:, h, :], start=(h == 0),
                                 stop=(h == H - 1))
        nc.vector.tensor_copy(out=out_sb[:, :], in_=out_ps)
        nc.sync.dma_start(out=out[:, :], in_=out_sb[:, :])


def make_host_constants(inputs):
    """Host-side prep of tables derived from the (full) inputs."""
    cos = np.asarray(inputs["cos"]).reshape(S, 64).astype(np.float32)
    sin = np.asarray(inputs["sin"]).reshape(S, 64).astype(np.float32)
    sink = np.asarray(inputs["sink"]).reshape(H, C).astype(np.float32)
    tao = np.asarray(inputs["tao"]).astype(np.float32)
    wq = np.asarray(inputs["W_qkvg"]).astype(np.float32)
    wo = np.asarray(inputs["W_out"]).astype(np.float32)

    pos = np.arange(64) + 1
    cos_p = cos[pos]
    sin_p = sin[pos]
    cosdup = np.tile(np.concatenate([cos_p, cos_p], axis=1), (2, 1))
    sinpm = np.tile(np.concatenate([sin_p, -sin_p], axis=1), (2, 1))
    taocol = np.concatenate([np.full((64, 1), tao[0], np.float32),
                             np.full((64, 1), tao[1], np.float32)])
    cosdup = (cosdup * taocol).astype(np.float16)
    sinpm = (sinpm * taocol).astype(np.float16)

    # additive causal mask TRANSPOSED: mask[t, s] = 0 if pos_t <= pos_s
    posf = np.where(np.arange(S) < NSEL, np.arange(S) + 1, 0)
    cmaskm = np.where(posf[:, None] <= posf[None, :], 0.0,
                      NEG_BIG).astype(np.float16)  # [t, s]

    sn = sink / np.sqrt((sink * sink).mean(axis=-1, keepdims=True) + EPS)
    sinkTq = np.ascontiguousarray((sn * tao[0]).T).astype(np.float16)
    sinkTk = np.ascontiguousarray((sn * tao[1]).T).astype(np.float16)
    sinkv = sink.reshape(1, H * C).astype(np.float16)

    # sel16'[p, r] = -16 if p == p(r); sel16'[4, r] = 16*NP + 4*T(r) + t(r)
    sel16m = np.zeros((5, NSEL), np.float32)
    for Tn in range(4):
        for p in range(4):
            for t in range(4):
                r = 16 * Tn + 4 * p + t
                sel16m[p, r] = -16.0
                sel16m[4, r] = float(16 * NP + 4 * Tn + t)

    negio = (float(NP) - np.arange(NP, dtype=np.float32)).astype(np.float16)

    wqT = np.ascontiguousarray(wq.T).astype(np.float16)
    woT = np.ascontiguousarray(
        wo.reshape(C, H, C).transpose(2, 1, 0)).astype(np.float16)

    identf32 = np.eye(128, dtype=np.float32)
    identf16 = np.eye(128, dtype=np.float16)

    def pack16(a, rows=128):
        c = a.shape[1]
        padded = np.zeros((128, c), np.float16)
        padded[:rows] = a
        return padded.view(np.float32)

    tabs2 = np.zeros((128, TB_COLS), np.float32)
    tabs2[0:5, TB_SEL16:TB_SEL16 + 64] = sel16m
    tabs2[:, TB_IDF16:TB_IDF16 + 64] = pack16(identf16)
    tabs2[:, TB_COS:TB_COS + 64] = pack16(cosdup)
    tabs2[:, TB_SIN:TB_SIN + 64] = pack16(sinpm)
    tabs2[:, TB_SINKQ:TB_SINKQ + 4] = pack16(sinkTq)
    tabs2[:, TB_SINKK:TB_SINKK + 4] = pack16(sinkTk)
    cm = np.zeros((65, 66), np.float16)
    cm[:, 0:65] = cmaskm
    tabs2[:, TB_CMASK:TB_CMASK + 33] = pack16(cm, rows=65)
    tabs2[:, TB_NEGIO:TB_NEGIO + 256] = pack16(negio.reshape(1, NP), rows=1)
    return dict(tabs2=tabs2, identd=identf32, sinkv=sinkv,
                wqT_d=wqT, woT_d=woT)


_CACHE = {}


def get_nc():
    if "nc" not in _CACHE:
        nc = bacc.Bacc("TRN2", target_bir_lowering=False, debug=False,
                       num_devices=B)
        build_kernel(nc)
        nc.compile()
        _CACHE["nc"] = nc
    return _CACHE["nc"]


def make_in_maps(inputs):
    x = np.ascontiguousarray(inputs["x"], dtype=np.float32)
    pwv = np.concatenate(
        [np.asarray(inputs["patch_w"], np.float32).ravel(),
         np.ones(128, np.float32)]).reshape(1, PATCH + 128)
    consts = make_host_constants(inputs)
    in_maps = []
    for b in range(B):
        m = {"xb": np.ascontiguousarray(x[b]), "pw": pwv}
        m.update(consts)
        in_maps.append(m)
    return in_maps


def kernel(**inputs):
    nc = get_nc()
    in_maps = make_in_maps(inputs)
    res = run_bass_kernel_spmd(nc, in_maps, core_ids=list(range(B)))
    return np.stack([r["out"] for r in res.results], axis=0)


if __name__ == "__main__":
    nc = get_nc()
    print("build ok:", len(nc.m.functions[0].allocations), "allocations")
